# revision 1
# baseline (speedup 1.0000x reference)
"""BiMamba2DFast kernel for 8 Trainium2 NeuronCores (Bass/Tile), v2.

Data-parallel over the 112 (b, w)-sequences - 14 per core. Per stage the
forward and reversed streams form one fused [128, 1568] token axis. All
matmuls run in fp16 (4x the fp32 PE rate; PSUM accumulates fp32). in_proj
runs on forward tokens only; the reversed stream's conv input is a
negative-stride DVE copy of the forward one. The causal depthwise conv runs
on the PE as three diagonal-matrix matmuls. B/C state rows are broadcast
across partitions by PE selection matmuls into PSUM, then cast to fp16
SBUF tiles by the scalar engine so the scan-loop multiplies hit the DVE
2x packed mode (a DRAM-loopback DMA broadcast was tried first: the
0-stride partition AP forces the software-dynamic DMA path, whose
sustained SBUF writes throttle concurrent DVE/GpSimd access ~2x). The selective scan is
fp16 tensor_tensor_scan (fp32 internal state), segmented by poisoning dt at
sequence starts. The n-sum of C*h: ct0/ct1 accumulate on the PE via
identity matmuls into held PSUM banks (seeded with diag(D) @ xc for the
skip term), ct2 accumulates on GpSimd; the C*h products (yt) run on GpSimd
and the B*dtxc products (dBx) on the DVE to balance engine load around the
DVE-resident scans.
"""
import sys

sys.path.insert(0, '/opt/trn_rl_repo')

import numpy as np
import concourse.bass as bass
import concourse.tile as tile
from concourse import mybir
from concourse.bass_utils import run_bass_kernel_spmd
import bass_rust

f32 = mybir.dt.float32
fp16 = mybir.dt.float16
Alu = mybir.AluOpType
Act = mybir.ActivationFunctionType

DM, DI, DS, DR = 192, 384, 16, 12
B, H, W, L = 2, 56, 56, 56
NSEQ = B * W                 # 112
NCORE = 8
SPC = NSEQ // NCORE          # 14 sequences per core
NT = SPC * L                 # 784 fwd tokens
T2 = 2 * NT                  # 1568 fwd+rev tokens
TC = 392
PADS = L + 2                 # 58 cols per seq in conv buffer
POISON = 30000.0

_nop_ctr = [0]


def _make_wait_nop(engine, wait):
    _nop_ctr[0] += 1
    inst = bass_rust.InstNoOp(name=f"waitnop-{_nop_ctr[0]}", hint="splitwait",
                              cycle_cnt=0)
    inst.engine = engine
    inst.sync_info = bass_rust.SyncInfo(on_wait=[wait], on_update=[])
    return inst


def split_excess_waits(nc, max_waits=1):
    """This walrus build rejects >max_waits sem waits per instruction; hoist
    the excess onto same-engine NoOps placed just before the instruction."""
    for fn in nc.m.functions:
        for bb in fn.blocks:
            if not any(inst.sync_info is not None and inst.sync_info.on_wait
                       and len(inst.sync_info.on_wait) > max_waits
                       for inst in bb.instructions):
                continue
            new_list = []
            for inst in bb.instructions:
                si = inst.sync_info
                if si is not None and si.on_wait and len(si.on_wait) > max_waits:
                    waits = list(si.on_wait)
                    keep = waits[-max_waits:]
                    for w in waits[:-max_waits]:
                        new_list.append(_make_wait_nop(inst.engine, w))
                    si.on_wait = keep
                new_list.append(inst)
            bb.instructions[:] = new_list


def build_nc():
    nc = bass.Bass()

    def din(nm, sh, dt=fp16):
        return nc.declare_dram_parameter(nm, list(sh), dt, isOutput=False)

    x1_d = din("x1", (128, NT))
    x2_d = din("x2", (64, NT))
    ident_d = din("ident", (128, 128))
    selB_d = din("selB", (DR + 2 * DS, DS * 128))
    selC_d = din("selC", (DR + 2 * DS, DS * 128))
    wt_dram = {}
    for p in ("h", "w"):
        wt_dram[p] = dict(
            inw1=din(f"{p}_inw1", (128, 2 * DI)),
            inw2=din(f"{p}_inw2", (64, 2 * DI)),
            xpw=din(f"{p}_xpw", (DI, DR + 2 * DS)),
            dtw=din(f"{p}_dtw", (DR, DI)),
            outw=din(f"{p}_outw", (DI, DM)),
            cdiag=din(f"{p}_cdiag", (DI, 3 * 128)),      # diag(conv_w[:,k]) blocks
            ddiag=din(f"{p}_ddiag", (DI, 128)),          # diag(D_skip)
            convb=din(f"{p}_convb", (DI, 1), f32),
            dtb=din(f"{p}_dtb", (DI, 1), f32),
            A=din(f"{p}_A", (DI, DS), f32),
        )
    out_d = nc.declare_dram_parameter("out", [DM, NT], f32, isOutput=True)

    with tile.TileContext(nc) as tc:
        with (
            tc.tile_pool(name="pers", bufs=1) as pers,
            tc.tile_pool(name="work", bufs=1) as work,
            tc.tile_pool(name="sw", bufs=2) as sw,
            tc.tile_pool(name="w1", bufs=1) as w1,
            tc.tile_pool(name="bc", bufs=3) as bcp,
            tc.tile_pool(name="mm", bufs=4, space=bass.MemorySpace.PSUM) as pmm,
            tc.tile_pool(name="acc", bufs=4, space=bass.MemorySpace.PSUM) as paccp,
            tc.tile_pool(name="dram", bufs=1, space="DRAM") as dpool,
        ):
            o1 = dpool.tile([SPC, DM, L], fp16, tag="o1")

            identsb = pers.tile([128, 128], fp16, tag="ident")
            nc.gpsimd.dma_start(identsb[:], ident_d[:])
            selB = pers.tile([DR + 2 * DS, DS * 128], fp16, tag="selB")
            nc.gpsimd.dma_start(selB[:], selB_d[:])
            selC = pers.tile([DR + 2 * DS, DS * 128], fp16, tag="selC")
            nc.gpsimd.dma_start(selC[:], selC_d[:])

            wts = {}
            for p in ("h", "w"):
                d = wt_dram[p]
                w = {}
                for nm in ("inw1", "inw2", "dtw"):
                    t = pers.tile(list(d[nm].shape), fp16, tag=f"{p}{nm}")
                    nc.gpsimd.dma_start(t[:], d[nm][:])
                    w[nm] = t
                for ct in range(3):
                    for nm, cols, dt in (("xpw", DR + 2 * DS, fp16),
                                         ("outw", DM, fp16),
                                         ("cdiag", 3 * 128, fp16),
                                         ("ddiag", 128, fp16),
                                         ("convb", 1, f32),
                                         ("dtb", 1, f32), ("A", DS, f32)):
                        t = pers.tile([128, cols], dt, tag=f"{p}{nm}{ct}")
                        nc.gpsimd.dma_start(t[:], d[nm][ct * 128:(ct + 1) * 128, :])
                        w[f"{nm}{ct}"] = t
                wts[p] = w

            # conv padded buffers [128, 14, 58]; zero the 2 pad cols once
            xi_pad = {}
            for dr in ("f", "r"):
                for ct in range(3):
                    t = pers.tile([128, SPC * PADS], fp16, tag=f"xp{dr}{ct}")
                    r3 = t[:].rearrange("p (s q) -> p s q", q=PADS)
                    nc.vector.memset(r3[:, :, 0:2], 0.0)
                    xi_pad[(dr, ct)] = t

            o1_flat = o1[:].rearrange("s d h -> s (d h)").rearrange(
                "s (i j) -> (s i) j", j=DM)

            def emit_stage(si, w):
                # ---- input (fwd tokens only) ----
                in1 = work.tile([128, NT], fp16, tag="in1")
                in2 = work.tile([64, NT], fp16, tag="in2")
                if si == 0:
                    nc.sync.dma_start(in1[:], x1_d[:])
                    nc.sync.dma_start(in2[:], x2_d[:])
                else:
                    for t in range(7):
                        x2f = sw.tile([112, DM], fp16, tag="x2f")
                        nc.sync.dma_start(x2f[:], o1_flat[t * 112:(t + 1) * 112, :])
                        pt1 = pmm.tile([128, 112], fp16, tag="mm")
                        nc.tensor.matmul(pt1[:], x2f[:, 0:128],
                                         identsb[0:112, 0:112],
                                         is_transpose=True, start=True, stop=True)
                        nc.scalar.copy(in1[:, t * 112:(t + 1) * 112], pt1[:])
                        pt2 = pmm.tile([64, 112], fp16, tag="mm")
                        nc.tensor.matmul(pt2[:], x2f[:, 128:192],
                                         identsb[0:112, 0:112],
                                         is_transpose=True, start=True, stop=True)
                        nc.scalar.copy(in2[:, t * 112:(t + 1) * 112], pt2[:])

                # ---- in_proj (fwd) ----
                siluz = []
                for e in range(6):
                    for j in range(2):
                        t0 = j * TC
                        pm = pmm.tile([128, TC], f32, tag="mm")
                        nc.tensor.matmul(pm[:], w["inw1"][:, e * 128:(e + 1) * 128],
                                         in1[:, t0:t0 + TC], start=True, stop=False)
                        nc.tensor.matmul(pm[:], w["inw2"][:, e * 128:(e + 1) * 128],
                                         in2[:, t0:t0 + TC], start=False, stop=True)
                        if e < 3:
                            r3 = xi_pad[("f", e)][:].rearrange(
                                "p (s q) -> p s q", q=PADS)
                            nc.scalar.copy(r3[:, 7 * j:7 * j + 7, 2:PADS],
                                           pm[:].rearrange("p (s q) -> p s q", q=L))
                        else:
                            if j == 0:
                                t = work.tile([128, NT], fp16, tag=f"siluz{e - 3}")
                                siluz.append(t)
                            nc.scalar.activation(siluz[e - 3][:, t0:t0 + TC],
                                                 pm[:], Act.Silu)

                # ---- reversed xi (negative-stride TT-bypass copy) ----
                for ct in range(3):
                    rf = xi_pad[("f", ct)][:].rearrange("p (s q) -> p s q", q=PADS)
                    rr = xi_pad[("r", ct)][:].rearrange("p (s q) -> p s q", q=PADS)
                    src = rf[:, :, 2:PADS][:, :, ::-1]
                    nc.vector.tensor_tensor(rr[:, :, 2:PADS], src, src, Alu.bypass)

                # ---- conv on PE (3 diagonal matmuls) + silu -> xcb ----
                xcb, dtsp, dtxc = [], [], []
                for ct in range(3):
                    xc = work.tile([128, T2], fp16, tag=f"xcb{ct}")
                    xcb.append(xc)
                    for di, dr in enumerate(("f", "r")):
                        r3 = xi_pad[(dr, ct)][:].rearrange("p (s q) -> p s q", q=PADS)
                        for j in range(2):
                            pm = pmm.tile([128, TC], f32, tag="mm")
                            pm3 = pm[:].rearrange("p (s l) -> p s l", l=L)
                            for k in range(3):
                                nc.tensor.matmul(
                                    pm3, w[f"cdiag{ct}"][:, k * 128:(k + 1) * 128],
                                    r3[:, 7 * j:7 * j + 7, k:k + L],
                                    start=(k == 0), stop=(k == 2))
                            nc.scalar.activation(
                                xc[:, di * NT + j * TC:di * NT + (j + 1) * TC],
                                pm[:], Act.Silu, bias=w[f"convb{ct}"][:, 0:1])

                # ---- x_proj -> dbl fp16 (SBUF + DRAM for broadcasts) ----
                dbl = work.tile([DR + 2 * DS, T2], fp16, tag="dbl")
                for j in range(4):
                    t0 = j * TC
                    pd = pmm.tile([DR + 2 * DS, TC], f32, tag="mm")
                    for ct in range(3):
                        nc.tensor.matmul(pd[:], w[f"xpw{ct}"][:],
                                         xcb[ct][:, t0:t0 + TC],
                                         start=(ct == 0), stop=(ct == 2))
                    nc.scalar.copy(dbl[:, t0:t0 + TC], pd[:])

                # ---- dt softplus (exp, ln) -> dtsp fp16; dtxc fp16; poison ----
                for ct in range(3):
                    tsp32 = w1.tile([128, T2], f32, tag="tsp32")
                    for j in range(4):
                        t0 = j * TC
                        pm = pmm.tile([128, TC], f32, tag="mm")
                        nc.tensor.matmul(pm[:], w["dtw"][:, ct * 128:(ct + 1) * 128],
                                         dbl[0:DR, t0:t0 + TC], start=True, stop=True)
                        nc.scalar.activation(tsp32[:, t0:t0 + TC], pm[:], Act.Exp,
                                             bias=w[f"dtb{ct}"][:, 0:1])
                    t = work.tile([128, T2], fp16, tag=f"dtsp{ct}")
                    nc.scalar.activation(t[:], tsp32[:], Act.Ln, bias=1.0)
                    dtsp.append(t)
                    tx = work.tile([128, T2], fp16, tag=f"dtxc{ct}")
                    nc.vector.tensor_tensor(tx[:], t[:], xcb[ct][:], Alu.mult)
                    dtxc.append(tx)
                    r3 = t[:].rearrange("p (s l) -> p s l", l=L)
                    nc.vector.memset(r3[:, :, 0:1], POISON)

                # ---- yacc accumulators ----
                # ct2 first (transient psum): gpsimd accumulator seeded with
                # diag(D) @ xc via psum -> Act copy. Must precede the 8
                # persistent ct0/ct1 banks or the pool round-robin wraps onto
                # live accumulators and deadlocks.
                yacc2 = work.tile([128, T2], fp16, tag="yacc2")
                for j in range(4):
                    a = pmm.tile([128, TC], f32, tag="mm")
                    nc.tensor.matmul(a[:], w["ddiag2"][:],
                                     xcb[2][:, j * TC:(j + 1) * TC],
                                     start=True, stop=True)
                    nc.scalar.copy(yacc2[:, j * TC:(j + 1) * TC], a[:])
                # ct1: DVE sbuf accumulator seeded via psum -> Act copy
                yacc1 = work.tile([128, T2], fp16, tag="yacc1")
                for j in range(4):
                    a = pmm.tile([128, TC], f32, tag="mm")
                    nc.tensor.matmul(a[:], w["ddiag1"][:],
                                     xcb[1][:, j * TC:(j + 1) * TC],
                                     start=True, stop=True)
                    nc.scalar.copy(yacc1[:, j * TC:(j + 1) * TC], a[:])
                # ct0: PE psum banks seeded with diag(D) @ xc
                acc = {}
                for j in range(4):
                    a = paccp.tile([128, TC], f32, tag="acc")
                    nc.tensor.matmul(a[:], w["ddiag0"][:],
                                     xcb[0][:, j * TC:(j + 1) * TC],
                                     start=True, stop=False)
                    acc[(0, j)] = a

                # ---- scan loop over state index n ----
                for n in range(DS):
                    pcs = bcp.tile([128, T2], fp16, tag="pcs")
                    for j in range(4):
                        t0 = j * TC
                        pb = pmm.tile([128, TC], f32, tag="mm")
                        nc.tensor.matmul(pb[:], selC[:, n * 128:(n + 1) * 128],
                                         dbl[:, t0:t0 + TC],
                                         start=True, stop=True)
                        nc.scalar.copy(pcs[:, t0:t0 + TC], pb[:])
                    pbs_ps = []
                    for j in range(4):
                        t0 = j * TC
                        pb = pmm.tile([128, TC], f32, tag="mm")
                        nc.tensor.matmul(pb[:], selB[:, n * 128:(n + 1) * 128],
                                         dbl[:, t0:t0 + TC],
                                         start=True, stop=True)
                        pbs_ps.append(pb)
                    for ct in range(3):
                        dA = sw.tile([128, T2], fp16, tag=f"dA{ct}")
                        nc.scalar.activation(dA[:], dtsp[ct][:], Act.Exp,
                                             scale=w[f"A{ct}"][:, n:n + 1])
                        dBx = sw.tile([128, T2], fp16, tag=f"dBx{ct}")
                        for j in range(4):
                            t0 = j * TC
                            nc.vector.tensor_tensor(dBx[:, t0:t0 + TC],
                                                    dtxc[ct][:, t0:t0 + TC],
                                                    pbs_ps[j][:], Alu.mult)
                        hs = sw.tile([128, T2], fp16, tag=f"hs{ct}")
                        nc.vector.tensor_tensor_scan(hs[:], dA[:], dBx[:],
                                                     0.0, Alu.mult, Alu.add)
                        yt = sw.tile([128, T2], fp16, tag=f"yt{ct}")
                        nc.gpsimd.tensor_tensor(yt[:], hs[:], pcs[:], Alu.mult)
                        if ct == 0:
                            for j in range(4):
                                nc.tensor.matmul(acc[(0, j)][:], identsb[:],
                                                 yt[:, j * TC:(j + 1) * TC],
                                                 start=False, stop=(n == DS - 1))
                        elif ct == 1:
                            nc.vector.tensor_tensor(yacc1[:], yacc1[:], yt[:],
                                                    Alu.add)
                        else:
                            nc.gpsimd.tensor_tensor(yacc2[:], yacc2[:], yt[:],
                                                    Alu.add)

                # ---- gate with silu(z), bidirectional sum ----
                gs = []
                for ct in range(3):
                    g = w1.tile([128, T2], fp16, tag=f"g{ct}")
                    sz3 = siluz[ct][:].rearrange("p (s l) -> p s l", l=L)
                    if ct == 0:
                        for j in range(4):
                            cols = slice(j * TC, (j + 1) * TC)
                            if j < 2:
                                nc.vector.tensor_tensor(
                                    g[:, cols], siluz[ct][:, cols], acc[(0, j)][:],
                                    Alu.mult)
                            else:
                                nc.vector.tensor_tensor(
                                    g[:, cols].rearrange("p (s l) -> p s l", l=L),
                                    sz3[:, 7 * (j - 2):7 * (j - 1), ::-1],
                                    acc[(0, j)][:].rearrange("p (s l) -> p s l", l=L),
                                    Alu.mult)
                    else:
                        ya = yacc1 if ct == 1 else yacc2
                        nc.vector.tensor_tensor(g[:, 0:NT], ya[:, 0:NT],
                                                siluz[ct][:], Alu.mult)
                        nc.vector.tensor_tensor(
                            g[:, NT:T2].rearrange("p (s l) -> p s l", l=L),
                            ya[:, NT:T2].rearrange("p (s l) -> p s l", l=L),
                            sz3[:, :, ::-1], Alu.mult)
                    gsum = w1.tile([128, NT], fp16, tag=f"gs{ct}")
                    nc.vector.tensor_tensor(gsum[:], g[:, 0:NT], g[:, NT:T2], Alu.add)
                    gs.append(gsum)

                # ---- out_proj ----
                for j in range(2):
                    for dchunk, dlo, dhi in ((0, 0, 128), (1, 128, 192)):
                        t0 = j * TC
                        po = pmm.tile([dhi - dlo, TC], f32, tag="mm")
                        for ct in range(3):
                            nc.tensor.matmul(po[:], w[f"outw{ct}"][:, dlo:dhi],
                                             gs[ct][:, t0:t0 + TC],
                                             start=(ct == 0), stop=(ct == 2))
                        if si == 0:
                            osb = w1.tile([dhi - dlo, TC], fp16, tag=f"osb{dchunk}")
                            nc.scalar.copy(osb[:], po[:])
                            dap = o1[:, dlo:dhi, :].transpose([1, 0, 2])
                            nc.sync.dma_start(
                                dap[:, 7 * j:7 * j + 7, :],
                                osb[:].rearrange("p (s l) -> p s l", l=L))
                        else:
                            osb = w1.tile([dhi - dlo, TC], f32, tag=f"osbf{dchunk}")
                            nc.scalar.copy(osb[:], po[:])
                            nc.sync.dma_start(out_d[dlo:dhi, t0:t0 + TC], osb[:])

            emit_stage(0, wts["h"])
            emit_stage(1, wts["w"])

    split_excess_waits(nc)
    return nc


_NC_CACHE = None


def _get_nc():
    global _NC_CACHE
    if _NC_CACHE is None:
        _NC_CACHE = build_nc()
    return _NC_CACHE


def build_in_maps(inputs):
    inputs = {k: np.asarray(v, dtype=np.float32) for k, v in inputs.items()}
    x = inputs["x"]
    h16 = lambda a: np.ascontiguousarray(a.astype(np.float16))

    selB = np.zeros((DR + 2 * DS, DS * 128), np.float16)
    selC = np.zeros((DR + 2 * DS, DS * 128), np.float16)
    for n in range(DS):
        selB[DR + n, n * 128:(n + 1) * 128] = 1.0
        selC[DR + DS + n, n * 128:(n + 1) * 128] = 1.0
    base = {"ident": np.eye(128, dtype=np.float16),
            "selB": selB, "selC": selC}
    for p, tag in (("h", "h_"), ("w", "w_")):
        inw = inputs[tag + "in_proj_w"].T                        # [192, 768]
        base[f"{p}_inw1"] = h16(inw[0:128, :])
        base[f"{p}_inw2"] = h16(inw[128:192, :])
        base[f"{p}_xpw"] = h16(inputs[tag + "x_proj_w"].T)
        base[f"{p}_dtw"] = h16(inputs[tag + "dt_proj_w"].T)
        base[f"{p}_outw"] = h16(inputs[tag + "out_proj_w"].T)
        cw = inputs[tag + "conv_w"]                              # [384, 3]
        cdiag = np.zeros((DI, 3 * 128), np.float16)
        for ct in range(3):
            for k in range(3):
                cdiag[ct * 128:(ct + 1) * 128, k * 128:(k + 1) * 128] = \
                    np.diag(cw[ct * 128:(ct + 1) * 128, k].astype(np.float16))
        base[f"{p}_cdiag"] = cdiag
        dsk = inputs[tag + "D_skip"]
        ddiag = np.zeros((DI, 128), np.float16)
        for ct in range(3):
            ddiag[ct * 128:(ct + 1) * 128, :] = \
                np.diag(dsk[ct * 128:(ct + 1) * 128].astype(np.float16))
        base[f"{p}_ddiag"] = ddiag
        base[f"{p}_convb"] = inputs[tag + "conv_b"].reshape(DI, 1).copy()
        base[f"{p}_dtb"] = inputs[tag + "dt_proj_b"].reshape(DI, 1).copy()
        base[f"{p}_A"] = np.ascontiguousarray(-np.exp(inputs[tag + "A_log"]))

    in_maps = []
    for core in range(NCORE):
        sl = range(core * SPC, (core + 1) * SPC)
        seqs = np.stack([x[s // W, :, s % W, :] for s in sl])     # [14, 56, 192]
        xt = np.ascontiguousarray(seqs.reshape(NT, DM).T)         # [192, 784]
        m = dict(base)
        m["x1"] = h16(xt[0:128, :])
        m["x2"] = h16(xt[128:192, :])
        in_maps.append(m)
    return in_maps


def kernel(**inputs):
    in_maps = build_in_maps(inputs)
    nc = _get_nc()
    res = run_bass_kernel_spmd(nc, in_maps, core_ids=list(range(NCORE)))

    out_full = np.zeros((NSEQ, L, DM), np.float32)
    for core in range(NCORE):
        o = np.asarray(res.results[core]["out"], dtype=np.float32)   # [192, 784]
        out_full[core * SPC:(core + 1) * SPC] = o.T.reshape(SPC, L, DM)
    return out_full.reshape(B, H, W, DM)



# revision 2
# speedup vs baseline: 1.0010x; 1.0010x over previous
"""BiMamba2DFast kernel for 8 Trainium2 NeuronCores (Bass/Tile), v3.

Data-parallel over the 112 (b, w)-sequences - 14 per core. Key changes vs v2:
GpSimd does no elementwise work (its TT ops share the DVE SBUF port and
roughly double both engines' per-op time - measured). The scan loop is
restructured into two 784-token halves (fwd stream, rev stream) so that all
THREE 128-channel blocks accumulate y over n on the PE into held PSUM banks
(6 acc banks + 2 broadcast banks = 8). B rows are cast to fp16 SBUF like C
so the dBx/yt multiplies run in the DVE 2x packed mode. The scan is fp16
tensor_tensor_scan (fp32 internal state), segmented by poisoning dt at
sequence starts.
"""
import sys

sys.path.insert(0, '/opt/trn_rl_repo')

import numpy as np
import concourse.bass as bass
import concourse.tile as tile
from concourse import mybir
from concourse.bass_utils import run_bass_kernel_spmd
import bass_rust

f32 = mybir.dt.float32
fp16 = mybir.dt.float16
Alu = mybir.AluOpType
Act = mybir.ActivationFunctionType

DM, DI, DS, DR = 192, 384, 16, 12
B, H, W, L = 2, 56, 56, 56
NSEQ = B * W                 # 112
NCORE = 8
SPC = NSEQ // NCORE          # 14 sequences per core
NT = SPC * L                 # 784 fwd tokens
T2 = 2 * NT                  # 1568 fwd+rev tokens
TC = 392
PADS = L + 2                 # 58 cols per seq in conv buffer
POISON = 30000.0

_nop_ctr = [0]


def _make_wait_nop(engine, wait):
    _nop_ctr[0] += 1
    inst = bass_rust.InstNoOp(name=f"waitnop-{_nop_ctr[0]}", hint="splitwait",
                              cycle_cnt=0)
    inst.engine = engine
    inst.sync_info = bass_rust.SyncInfo(on_wait=[wait], on_update=[])
    return inst


def split_excess_waits(nc, max_waits=1):
    """This walrus build rejects >max_waits sem waits per instruction; hoist
    the excess onto same-engine NoOps placed just before the instruction."""
    for fn in nc.m.functions:
        for bb in fn.blocks:
            if not any(inst.sync_info is not None and inst.sync_info.on_wait
                       and len(inst.sync_info.on_wait) > max_waits
                       for inst in bb.instructions):
                continue
            new_list = []
            for inst in bb.instructions:
                si = inst.sync_info
                if si is not None and si.on_wait and len(si.on_wait) > max_waits:
                    waits = list(si.on_wait)
                    keep = waits[-max_waits:]
                    for w in waits[:-max_waits]:
                        new_list.append(_make_wait_nop(inst.engine, w))
                    si.on_wait = keep
                new_list.append(inst)
            bb.instructions[:] = new_list


def build_nc():
    nc = bass.Bass()

    def din(nm, sh, dt=fp16):
        return nc.declare_dram_parameter(nm, list(sh), dt, isOutput=False)

    x1_d = din("x1", (128, NT))
    x2_d = din("x2", (64, NT))
    ident_d = din("ident", (128, 128))
    selB_d = din("selB", (DR + 2 * DS, DS * 128))
    selC_d = din("selC", (DR + 2 * DS, DS * 128))
    wt_dram = {}
    for p in ("h", "w"):
        wt_dram[p] = dict(
            inw1=din(f"{p}_inw1", (128, 2 * DI)),
            inw2=din(f"{p}_inw2", (64, 2 * DI)),
            xpw=din(f"{p}_xpw", (DI, DR + 2 * DS)),
            dtw=din(f"{p}_dtw", (DR, DI)),
            outw=din(f"{p}_outw", (DI, DM)),
            cdiag=din(f"{p}_cdiag", (DI, 3 * 128)),      # diag(conv_w[:,k]) blocks
            ddiag=din(f"{p}_ddiag", (DI, 128)),          # diag(D_skip)
            convb=din(f"{p}_convb", (DI, 1), f32),
            dtb=din(f"{p}_dtb", (DI, 1), f32),
            A=din(f"{p}_A", (DI, DS), f32),
        )
    out_d = nc.declare_dram_parameter("out", [DM, NT], f32, isOutput=True)

    with tile.TileContext(nc) as tc:
        with (
            tc.tile_pool(name="pers", bufs=1) as pers,
            tc.tile_pool(name="work", bufs=1) as work,
            tc.tile_pool(name="sw", bufs=2) as sw,
            tc.tile_pool(name="w1", bufs=1) as w1,
            tc.tile_pool(name="bc", bufs=2) as bcp,
            tc.tile_pool(name="mm", bufs=2, space=bass.MemorySpace.PSUM) as pmm,
            tc.tile_pool(name="acc", bufs=6, space=bass.MemorySpace.PSUM) as paccp,
            tc.tile_pool(name="dram", bufs=1, space="DRAM") as dpool,
        ):
            # o1 split by 7-seq halves so stage 2's readback of the first
            # sequences can start while stage 1 still writes the second half
            o1a = dpool.tile([SPC // 2, DM, L], fp16, tag="o1a")
            o1b = dpool.tile([SPC // 2, DM, L], fp16, tag="o1b")

            # weight-load DMAs: issue order matters (~650ns of issue time
            # each, serialized per queue). Load stage-h weights in pipeline
            # order on gpsimd+sync (scalar stays free for phase-A ACT work);
            # stage-w weights go last - they aren't needed for ~300us.
            _ectr = [0]

            def wload(dst, src):
                e = nc.gpsimd if _ectr[0] % 2 == 0 else nc.sync
                _ectr[0] += 1
                e.dma_start(dst, src)

            wts = {}
            load2 = []
            for p in ("h", "w"):
                d = wt_dram[p]
                w = {}
                for nm in ("inw1", "inw2", "dtw"):
                    t = pers.tile(list(d[nm].shape), fp16, tag=f"{p}{nm}")
                    if p == "h" and nm != "dtw":
                        wload(t[:], d[nm][:])
                    else:
                        load2.append((t[:], d[nm][:]))
                    w[nm] = t
                for ct in range(3):
                    for nm, cols, dt in (("cdiag", 3 * 128, fp16),
                                         ("convb", 1, f32),
                                         ("xpw", DR + 2 * DS, fp16),
                                         ("dtb", 1, f32), ("A", DS, f32),
                                         ("ddiag", 128, fp16),
                                         ("outw", DM, fp16)):
                        t = pers.tile([128, cols], dt, tag=f"{p}{nm}{ct}")
                        if p == "h" and nm in ("cdiag", "convb"):
                            wload(t[:], d[nm][ct * 128:(ct + 1) * 128, :])
                        else:
                            load2.append((t[:], d[nm][ct * 128:(ct + 1) * 128, :]))
                        w[f"{nm}{ct}"] = t
                wts[p] = w

            identsb = pers.tile([128, 128], fp16, tag="ident")
            wload(identsb[:], ident_d[:])
            selB = pers.tile([DR + 2 * DS, DS * 128], fp16, tag="selB")
            wload(selB[:], selB_d[:])
            selC = pers.tile([DR + 2 * DS, DS * 128], fp16, tag="selC")
            wload(selC[:], selC_d[:])
            for dst, src in load2:
                wload(dst, src)

            # conv padded buffers [128, 14, 58]; zero the 2 pad cols once
            xi_pad = {}
            for dr in ("f", "r"):
                for ct in range(3):
                    t = pers.tile([128, SPC * PADS], fp16, tag=f"xp{dr}{ct}")
                    r3 = t[:].rearrange("p (s q) -> p s q", q=PADS)
                    nc.vector.memset(r3[:, :, 0:2], 0.0)
                    xi_pad[(dr, ct)] = t

            o1f = [
                t[:].rearrange("s d h -> s (d h)").rearrange(
                    "s (i j) -> (s i) j", j=DM) for t in (o1a, o1b)]

            def emit_stage(si, w):
                # ---- input (fwd tokens only) ----
                in1 = work.tile([128, NT], fp16, tag="in1")
                in2 = work.tile([64, NT], fp16, tag="in2")
                if si == 0:
                    # scalar queue is DMA-free at t=0; these must land first
                    nc.scalar.dma_start(in1[:], x1_d[:])
                    nc.scalar.dma_start(in2[:], x2_d[:])
                else:
                    for t in range(7):
                        x2f = sw.tile([112, DM], fp16, tag="x2f")
                        r0, r1 = t * 112, (t + 1) * 112
                        if r1 <= 392:
                            nc.sync.dma_start(x2f[:], o1f[0][r0:r1, :])
                        elif r0 >= 392:
                            nc.sync.dma_start(x2f[:], o1f[1][r0 - 392:r1 - 392, :])
                        else:
                            nc.sync.dma_start(x2f[0:392 - r0, :],
                                              o1f[0][r0:392, :])
                            nc.sync.dma_start(x2f[392 - r0:112, :],
                                              o1f[1][0:r1 - 392, :])
                        pt1 = pmm.tile([128, 112], fp16, tag="mm")
                        nc.tensor.matmul(pt1[:], x2f[:, 0:128],
                                         identsb[0:112, 0:112],
                                         is_transpose=True, start=True, stop=True)
                        nc.vector.tensor_scalar_add(
                            in1[:, t * 112:(t + 1) * 112], pt1[:], 0.0)
                        pt2 = pmm.tile([64, 112], fp16, tag="mm")
                        nc.tensor.matmul(pt2[:], x2f[:, 128:192],
                                         identsb[0:112, 0:112],
                                         is_transpose=True, start=True, stop=True)
                        nc.vector.tensor_scalar_add(
                            in2[:, t * 112:(t + 1) * 112], pt2[:], 0.0)

                # ---- in_proj helpers: the xi part (e<3) is head-critical,
                # the z part (silu gate, needed only at gating) is deferred
                # into the jc0 n-loop ----
                siluz = []
                for e in range(3, 6):
                    t = work.tile([128, NT], fp16, tag=f"siluz{e - 3}")
                    siluz.append(t)

                def in_proj_unit(e, j, pool, ptag):
                    t0 = j * TC
                    pm = pool.tile([128, TC], f32, tag=ptag, name="pm")
                    nc.tensor.matmul(pm[:], w["inw1"][:, e * 128:(e + 1) * 128],
                                     in1[:, t0:t0 + TC], start=True, stop=False)
                    nc.tensor.matmul(pm[:], w["inw2"][:, e * 128:(e + 1) * 128],
                                     in2[:, t0:t0 + TC], start=False, stop=True)
                    if e < 3:
                        # copy on DVE - it idles in phase A
                        r3 = xi_pad[("f", e)][:].rearrange(
                            "p (s q) -> p s q", q=PADS)
                        nc.vector.tensor_scalar_add(
                            r3[:, 7 * j:7 * j + 7, 2:PADS],
                            pm[:].rearrange("p (s q) -> p s q", q=L), 0.0)
                    else:
                        nc.scalar.activation(siluz[e - 3][:, t0:t0 + TC],
                                             pm[:], Act.Silu)

                def rev_xi(ct):
                    rf = xi_pad[("f", ct)][:].rearrange("p (s q) -> p s q", q=PADS)
                    rr = xi_pad[("r", ct)][:].rearrange("p (s q) -> p s q", q=PADS)
                    src = rf[:, :, 2:PADS][:, :, ::-1]
                    nc.vector.tensor_tensor(rr[:, :, 2:PADS], src, src, Alu.bypass)

                # ---- conv / x_proj / dt helpers; fwd half emitted up-front,
                # rev half interleaved into the jc0 n-loop so its PE/Scalar
                # work hides under the DVE-bound scans (PE queues are
                # in-order, so emission order = PE execution order) ----
                xcb, dtsp, dtxc = [], [], []
                for ct in range(3):
                    xc = work.tile([128, T2], fp16, tag=f"xcb{ct}")
                    xcb.append(xc)
                    t = work.tile([128, T2], fp16, tag=f"dtsp{ct}")
                    dtsp.append(t)
                    tx = work.tile([128, T2], fp16, tag=f"dtxc{ct}")
                    dtxc.append(tx)
                dbl = work.tile([DR + 2 * DS, T2], fp16, tag="dbl")
                tsp32 = w1.tile([128, 3 * T2], f32, tag="tsp32")

                def conv_group(di, j, ct, pool=None, ptag=None):
                    # rev half (di=1) runs inside the jc0 n-loop while all 6
                    # "acc" banks are held -> must draw from the "mm" ring
                    if pool is None:
                        pool, ptag = (paccp, "acc") if di == 0 else (pmm, "mm")
                    dr = "fr"[di]
                    r3 = xi_pad[(dr, ct)][:].rearrange("p (s q) -> p s q", q=PADS)
                    pm = pool.tile([128, TC], f32, tag=ptag, name="pm")
                    pm3 = pm[:].rearrange("p (s l) -> p s l", l=L)
                    for k in range(3):
                        nc.tensor.matmul(
                            pm3, w[f"cdiag{ct}"][:, k * 128:(k + 1) * 128],
                            r3[:, 7 * j:7 * j + 7, k:k + L],
                            start=(k == 0), stop=(k == 2))
                    nc.scalar.activation(
                        xcb[ct][:, di * NT + j * TC:di * NT + (j + 1) * TC],
                        pm[:], Act.Silu, bias=w[f"convb{ct}"][:, 0:1])

                def xproj(j, pool, ptag):
                    t0 = j * TC
                    pd = pool.tile([DR + 2 * DS, TC], f32, tag=ptag, name="pd")
                    for ct in range(3):
                        nc.tensor.matmul(pd[:], w[f"xpw{ct}"][:],
                                         xcb[ct][:, t0:t0 + TC],
                                         start=(ct == 0), stop=(ct == 2))
                    nc.vector.tensor_scalar_add(dbl[:, t0:t0 + TC], pd[:], 0.0)

                def dt_unit(j, ct, pool, ptag):
                    t0 = j * TC
                    pm = pool.tile([128, TC], f32, tag=ptag, name="pm")
                    nc.tensor.matmul(pm[:], w["dtw"][:, ct * 128:(ct + 1) * 128],
                                     dbl[0:DR, t0:t0 + TC], start=True, stop=True)
                    nc.scalar.activation(tsp32[:, ct * T2 + t0:ct * T2 + t0 + TC],
                                         pm[:], Act.Exp,
                                         bias=w[f"dtb{ct}"][:, 0:1])
                    nc.scalar.activation(dtsp[ct][:, t0:t0 + TC],
                                         tsp32[:, ct * T2 + t0:ct * T2 + t0 + TC],
                                         Act.Ln, bias=1.0)

                def xproj_dt(j):
                    xproj(j, paccp, "acc")
                    for ct in range(3):
                        dt_unit(j, ct, paccp, "acc")

                def dtxc_ct(jc, ct):
                    jO = jc * NT
                    nc.vector.tensor_tensor(dtxc[ct][:, jO:jO + NT],
                                            dtsp[ct][:, jO:jO + NT],
                                            xcb[ct][:, jO:jO + NT], Alu.mult)
                    r3 = dtsp[ct][:, jO:jO + NT].rearrange(
                        "p (s l) -> p s l", l=L)
                    nc.vector.memset(r3[:, :, 0:1], POISON)

                def dtxc_half(jc):
                    for ct in range(3):
                        dtxc_ct(jc, ct)

                # head: only what the jc0 loop needs - in_proj-xi + conv-fwd
                # + x_proj/dt fwd chunks. z-projection and the whole rev half
                # are deferred into the jc0 n-loop.
                for j in range(2):
                    for e in range(3):
                        in_proj_unit(e, j, paccp, "acc")
                    for ct in range(3):
                        conv_group(0, j, ct)
                for ct in range(3):
                    rev_xi(ct)
                for j in range(2):
                    xproj_dt(j)
                dtxc_half(0)

                # deferred work units, emitted inside the jc0 n-loop (after
                # the ct0 block so they don't delay dA0). n=2..7 carry one
                # rev-conv + one z-projection unit; n=8..15 carry the rev
                # x_proj / dt chain so the jc0->jc1 transition is short.
                sched = {}
                for i, (jj, ct) in enumerate([(j, ct) for j in range(2)
                                              for ct in range(3)]):
                    sched.setdefault(2 + i, []).append(
                        lambda j=jj, c=ct: conv_group(1, j, c))
                for i, (jj, e) in enumerate([(j, e) for j in range(2)
                                             for e in range(3, 6)]):
                    sched.setdefault(2 + i, []).append(
                        lambda e=e, j=jj: in_proj_unit(e, j, pmm, "mm"))
                sched[8] = [lambda: xproj(2, pmm, "mm")]
                sched[9] = [lambda: dt_unit(2, 0, pmm, "mm")]
                sched[10] = [lambda: dt_unit(2, 1, pmm, "mm")]
                sched[11] = [lambda: dt_unit(2, 2, pmm, "mm")]
                sched[12] = [lambda: xproj(3, pmm, "mm")]
                sched[13] = [lambda: dt_unit(3, 0, pmm, "mm")]
                sched[14] = [lambda: dt_unit(3, 1, pmm, "mm"),
                             lambda: dtxc_ct(1, 0)]
                sched[15] = [lambda: dt_unit(3, 2, pmm, "mm"),
                             lambda: dtxc_ct(1, 1)]

                # ---- scan loop: two 784-token halves (fwd, rev) ----
                gsum, grs = [], []
                for ct in range(3):
                    g = work.tile([128, NT], fp16, tag=f"gs{ct}")
                    gsum.append(g)
                for jc in range(2):
                    jO = jc * NT
                    # 6 held acc banks, seeded with diag(D) @ xc
                    acc = {}
                    for ct in range(3):
                        for q in range(2):
                            a = paccp.tile([128, TC], f32, tag="acc", name="a")
                            nc.tensor.matmul(
                                a[:], w[f"ddiag{ct}"][:],
                                xcb[ct][:, jO + q * TC:jO + (q + 1) * TC],
                                start=True, stop=False)
                            acc[(ct, q)] = a
                    for n in range(DS):
                        pc_sb = bcp.tile([128, NT], fp16, tag="pc")
                        pb_sb = bcp.tile([128, NT], fp16, tag="pb")
                        for q in range(2):
                            t0 = jO + q * TC
                            pb = pmm.tile([128, TC], f32, tag="mm")
                            nc.tensor.matmul(pb[:], selC[:, n * 128:(n + 1) * 128],
                                             dbl[:, t0:t0 + TC],
                                             start=True, stop=True)
                            nc.scalar.copy(pc_sb[:, q * TC:(q + 1) * TC], pb[:])
                        for q in range(2):
                            t0 = jO + q * TC
                            pb = pmm.tile([128, TC], f32, tag="mm")
                            nc.tensor.matmul(pb[:], selB[:, n * 128:(n + 1) * 128],
                                             dbl[:, t0:t0 + TC],
                                             start=True, stop=True)
                            nc.scalar.copy(pb_sb[:, q * TC:(q + 1) * TC], pb[:])
                        for ct in range(3):
                            dA = sw.tile([128, NT], fp16, tag=f"dA{ct}")
                            nc.scalar.activation(dA[:], dtsp[ct][:, jO:jO + NT],
                                                 Act.Exp,
                                                 scale=w[f"A{ct}"][:, n:n + 1])
                            dBx = sw.tile([128, NT], fp16, tag=f"dBx{ct}")
                            nc.vector.tensor_tensor(dBx[:], dtxc[ct][:, jO:jO + NT],
                                                    pb_sb[:], Alu.mult)
                            hs = sw.tile([128, NT], fp16, tag=f"hs{ct}")
                            nc.vector.tensor_tensor_scan(hs[:], dA[:], dBx[:],
                                                         0.0, Alu.mult, Alu.add)
                            yt = sw.tile([128, NT], fp16, tag=f"yt{ct}")
                            nc.vector.tensor_tensor(yt[:], hs[:], pc_sb[:], Alu.mult)
                            for q in range(2):
                                nc.tensor.matmul(acc[(ct, q)][:], identsb[:],
                                                 yt[:, q * TC:(q + 1) * TC],
                                                 start=False, stop=(n == DS - 1))
                            if ct == 0 and jc == 0:
                                for f in sched.get(n, ()):
                                    f()
                    # ---- gate this half with silu(z) ----
                    for ct in range(3):
                        sz3 = siluz[ct][:].rearrange("p (s l) -> p s l", l=L)
                        if jc == 0:
                            for q in range(2):
                                nc.vector.tensor_tensor(
                                    gsum[ct][:, q * TC:(q + 1) * TC],
                                    siluz[ct][:, q * TC:(q + 1) * TC],
                                    acc[(ct, q)][:], Alu.mult)
                        else:
                            gr = sw.tile([128, NT], fp16, tag=f"gr{ct}")
                            gr3 = gr[:].rearrange("p (s l) -> p s l", l=L)
                            for q in range(2):
                                nc.vector.tensor_tensor(
                                    gr3[:, 7 * q:7 * q + 7, :],
                                    sz3[:, 7 * q:7 * q + 7, ::-1],
                                    acc[(ct, q)][:].rearrange(
                                        "p (s l) -> p s l", l=L),
                                    Alu.mult)
                            grs.append(gr)
                    if jc == 0:
                        dtxc_ct(1, 2)

                # ---- bidirectional sum + out_proj, chunked so the first
                # out chunk (and stage-2's readback) starts asap ----
                for j in range(2):
                    t0 = j * TC
                    for ct in range(3):
                        nc.vector.tensor_tensor(gsum[ct][:, t0:t0 + TC],
                                                gsum[ct][:, t0:t0 + TC],
                                                grs[ct][:, t0:t0 + TC], Alu.add)
                    for dchunk, dlo, dhi in ((0, 0, 128), (1, 128, 192)):
                        po = pmm.tile([dhi - dlo, TC], f32, tag="mm")
                        for ct in range(3):
                            nc.tensor.matmul(po[:], w[f"outw{ct}"][:, dlo:dhi],
                                             gsum[ct][:, t0:t0 + TC],
                                             start=(ct == 0), stop=(ct == 2))
                        if si == 0:
                            osb = w1.tile([dhi - dlo, TC], fp16, tag=f"osb{dchunk}")
                            nc.scalar.copy(osb[:], po[:])
                            o1x = o1a if j == 0 else o1b
                            dap = o1x[:, dlo:dhi, :].transpose([1, 0, 2])
                            nc.sync.dma_start(
                                dap[:, :, :],
                                osb[:].rearrange("p (s l) -> p s l", l=L))
                        else:
                            osb = w1.tile([dhi - dlo, TC], f32, tag=f"osbf{dchunk}")
                            nc.scalar.copy(osb[:], po[:])
                            nc.sync.dma_start(out_d[dlo:dhi, t0:t0 + TC], osb[:])

            emit_stage(0, wts["h"])
            emit_stage(1, wts["w"])

    split_excess_waits(nc)
    return nc


_NC_CACHE = None


def _get_nc():
    global _NC_CACHE
    if _NC_CACHE is None:
        _NC_CACHE = build_nc()
    return _NC_CACHE


def build_in_maps(inputs):
    inputs = {k: np.asarray(v, dtype=np.float32) for k, v in inputs.items()}
    x = inputs["x"]
    h16 = lambda a: np.ascontiguousarray(a.astype(np.float16))

    selB = np.zeros((DR + 2 * DS, DS * 128), np.float16)
    selC = np.zeros((DR + 2 * DS, DS * 128), np.float16)
    for n in range(DS):
        selB[DR + n, n * 128:(n + 1) * 128] = 1.0
        selC[DR + DS + n, n * 128:(n + 1) * 128] = 1.0
    base = {"ident": np.eye(128, dtype=np.float16),
            "selB": selB, "selC": selC}
    for p, tag in (("h", "h_"), ("w", "w_")):
        inw = inputs[tag + "in_proj_w"].T                        # [192, 768]
        base[f"{p}_inw1"] = h16(inw[0:128, :])
        base[f"{p}_inw2"] = h16(inw[128:192, :])
        base[f"{p}_xpw"] = h16(inputs[tag + "x_proj_w"].T)
        base[f"{p}_dtw"] = h16(inputs[tag + "dt_proj_w"].T)
        base[f"{p}_outw"] = h16(inputs[tag + "out_proj_w"].T)
        cw = inputs[tag + "conv_w"]                              # [384, 3]
        cdiag = np.zeros((DI, 3 * 128), np.float16)
        for ct in range(3):
            for k in range(3):
                cdiag[ct * 128:(ct + 1) * 128, k * 128:(k + 1) * 128] = \
                    np.diag(cw[ct * 128:(ct + 1) * 128, k].astype(np.float16))
        base[f"{p}_cdiag"] = cdiag
        dsk = inputs[tag + "D_skip"]
        ddiag = np.zeros((DI, 128), np.float16)
        for ct in range(3):
            ddiag[ct * 128:(ct + 1) * 128, :] = \
                np.diag(dsk[ct * 128:(ct + 1) * 128].astype(np.float16))
        base[f"{p}_ddiag"] = ddiag
        base[f"{p}_convb"] = inputs[tag + "conv_b"].reshape(DI, 1).copy()
        base[f"{p}_dtb"] = inputs[tag + "dt_proj_b"].reshape(DI, 1).copy()
        base[f"{p}_A"] = np.ascontiguousarray(-np.exp(inputs[tag + "A_log"]))

    in_maps = []
    for core in range(NCORE):
        sl = range(core * SPC, (core + 1) * SPC)
        seqs = np.stack([x[s // W, :, s % W, :] for s in sl])     # [14, 56, 192]
        xt = np.ascontiguousarray(seqs.reshape(NT, DM).T)         # [192, 784]
        m = dict(base)
        m["x1"] = h16(xt[0:128, :])
        m["x2"] = h16(xt[128:192, :])
        in_maps.append(m)
    return in_maps


def kernel(**inputs):
    in_maps = build_in_maps(inputs)
    nc = _get_nc()
    res = run_bass_kernel_spmd(nc, in_maps, core_ids=list(range(NCORE)))

    out_full = np.zeros((NSEQ, L, DM), np.float32)
    for core in range(NCORE):
        o = np.asarray(res.results[core]["out"], dtype=np.float32)   # [192, 784]
        out_full[core * SPC:(core + 1) * SPC] = o.T.reshape(SPC, L, DM)
    return out_full.reshape(B, H, W, DM)


# revision 4
# speedup vs baseline: 1.0368x; 1.0358x over previous
"""BiMamba2DFast kernel for 8 Trainium2 NeuronCores (Bass/Tile), v3.

Data-parallel over the 112 (b, w)-sequences - 14 per core. Key changes vs v2:
GpSimd does no elementwise work (its TT ops share the DVE SBUF port and
roughly double both engines' per-op time - measured). The scan loop is
restructured into two 784-token halves (fwd stream, rev stream) so that all
THREE 128-channel blocks accumulate y over n on the PE into held PSUM banks
(6 acc banks + 2 broadcast banks = 8). B rows are cast to fp16 SBUF like C
so the dBx/yt multiplies run in the DVE 2x packed mode. The scan is fp16
tensor_tensor_scan (fp32 internal state), segmented by poisoning dt at
sequence starts.
"""
import sys

sys.path.insert(0, '/opt/trn_rl_repo')

import numpy as np
import concourse.bass as bass
import concourse.tile as tile
from concourse import mybir
from concourse.bass_utils import run_bass_kernel_spmd
import bass_rust

f32 = mybir.dt.float32
fp16 = mybir.dt.float16
Alu = mybir.AluOpType
Act = mybir.ActivationFunctionType

DM, DI, DS, DR = 192, 384, 16, 12
B, H, W, L = 2, 56, 56, 56
NSEQ = B * W                 # 112
NCORE = 8
SPC = NSEQ // NCORE          # 14 sequences per core
NT = SPC * L                 # 784 fwd tokens
T2 = 2 * NT                  # 1568 fwd+rev tokens
TC = 392
PADS = L + 2                 # 58 cols per seq in conv buffer
POISON = 30000.0

_nop_ctr = [0]


def _make_wait_nop(engine, wait):
    _nop_ctr[0] += 1
    inst = bass_rust.InstNoOp(name=f"waitnop-{_nop_ctr[0]}", hint="splitwait",
                              cycle_cnt=0)
    inst.engine = engine
    inst.sync_info = bass_rust.SyncInfo(on_wait=[wait], on_update=[])
    return inst


def split_excess_waits(nc, max_waits=1):
    """This walrus build rejects >max_waits sem waits per instruction; hoist
    the excess onto same-engine NoOps placed just before the instruction."""
    for fn in nc.m.functions:
        for bb in fn.blocks:
            if not any(inst.sync_info is not None and inst.sync_info.on_wait
                       and len(inst.sync_info.on_wait) > max_waits
                       for inst in bb.instructions):
                continue
            new_list = []
            for inst in bb.instructions:
                si = inst.sync_info
                if si is not None and si.on_wait and len(si.on_wait) > max_waits:
                    waits = list(si.on_wait)
                    keep = waits[-max_waits:]
                    for w in waits[:-max_waits]:
                        new_list.append(_make_wait_nop(inst.engine, w))
                    si.on_wait = keep
                new_list.append(inst)
            bb.instructions[:] = new_list


def build_nc():
    nc = bass.Bass()

    def din(nm, sh, dt=fp16):
        return nc.declare_dram_parameter(nm, list(sh), dt, isOutput=False)

    x1_d = din("x1", (128, NT))
    x2_d = din("x2", (64, NT))
    ident_d = din("ident", (128, 128))
    selB_d = din("selB", (DR + 2 * DS, DS * 128))
    selC_d = din("selC", (DR + 2 * DS, DS * 128))
    wt_dram = {}
    for p in ("h", "w"):
        wt_dram[p] = dict(
            inw1=din(f"{p}_inw1", (128, 2 * DI)),
            inw2=din(f"{p}_inw2", (64, 2 * DI)),
            xpw=din(f"{p}_xpw", (DI, DR + 2 * DS)),
            dtw=din(f"{p}_dtw", (DR, DI)),
            outw=din(f"{p}_outw", (DI, DM)),
            cdiag=din(f"{p}_cdiag", (DI, 3 * 128)),      # diag(conv_w[:,k]) blocks
            ddiag=din(f"{p}_ddiag", (DI, 128)),          # diag(D_skip)
            convb=din(f"{p}_convb", (DI, 1), f32),
            dtb=din(f"{p}_dtb", (DI, 1), f32),
            A=din(f"{p}_A", (DI, DS), f32),
        )
    out_d = nc.declare_dram_parameter("out", [DM, NT], f32, isOutput=True)

    with tile.TileContext(nc) as tc:
        with (
            tc.tile_pool(name="pers", bufs=1) as pers,
            tc.tile_pool(name="work", bufs=1) as work,
            tc.tile_pool(name="sw", bufs=3) as sw,
            tc.tile_pool(name="w1", bufs=1) as w1,
            tc.tile_pool(name="bc", bufs=2) as bcp,
            tc.tile_pool(name="mm", bufs=2, space=bass.MemorySpace.PSUM) as pmm,
            tc.tile_pool(name="acc", bufs=6, space=bass.MemorySpace.PSUM) as paccp,
            tc.tile_pool(name="dram", bufs=1, space="DRAM") as dpool,
        ):
            # o1 split by 7-seq halves so stage 2's readback of the first
            # sequences can start while stage 1 still writes the second half
            o1a = dpool.tile([SPC // 2, DM, L], fp16, tag="o1a")
            o1b = dpool.tile([SPC // 2, DM, L], fp16, tag="o1b")

            # weight-load DMAs: issue order matters (~650ns of issue time
            # each, serialized per queue). Load stage-h weights in pipeline
            # order on gpsimd+sync (scalar stays free for phase-A ACT work);
            # stage-w weights go last - they aren't needed for ~300us.
            _ectr = [0]

            def wload(dst, src):
                e = nc.gpsimd if _ectr[0] % 2 == 0 else nc.sync
                _ectr[0] += 1
                e.dma_start(dst, src)

            wts = {}
            load2 = []
            for p in ("h", "w"):
                d = wt_dram[p]
                w = {}
                for nm in ("inw1", "inw2", "dtw"):
                    t = pers.tile(list(d[nm].shape), fp16, tag=f"{p}{nm}")
                    if p == "h" and nm != "dtw":
                        wload(t[:], d[nm][:])
                    else:
                        load2.append((t[:], d[nm][:]))
                    w[nm] = t
                for ct in range(3):
                    for nm, cols, dt in (("cdiag", 3 * 128, fp16),
                                         ("convb", 1, f32),
                                         ("xpw", DR + 2 * DS, fp16),
                                         ("dtb", 1, f32), ("A", DS, f32),
                                         ("ddiag", 128, fp16),
                                         ("outw", DM, fp16)):
                        t = pers.tile([128, cols], dt, tag=f"{p}{nm}{ct}")
                        if p == "h" and nm in ("cdiag", "convb"):
                            wload(t[:], d[nm][ct * 128:(ct + 1) * 128, :])
                        else:
                            load2.append((t[:], d[nm][ct * 128:(ct + 1) * 128, :]))
                        w[f"{nm}{ct}"] = t
                wts[p] = w

            identsb = pers.tile([128, 128], fp16, tag="ident")
            wload(identsb[:], ident_d[:])
            selB = pers.tile([DR + 2 * DS, DS * 128], fp16, tag="selB")
            wload(selB[:], selB_d[:])
            selC = pers.tile([DR + 2 * DS, DS * 128], fp16, tag="selC")
            wload(selC[:], selC_d[:])
            for dst, src in load2:
                wload(dst, src)

            # conv padded buffers [128, 14, 58]; zero the 2 pad cols once
            xi_pad = {}
            for dr in ("f", "r"):
                for ct in range(3):
                    t = pers.tile([128, SPC * PADS], fp16, tag=f"xp{dr}{ct}")
                    r3 = t[:].rearrange("p (s q) -> p s q", q=PADS)
                    nc.vector.memset(r3[:, :, 0:2], 0.0)
                    xi_pad[(dr, ct)] = t

            o1f = [
                t[:].rearrange("s d h -> s (d h)").rearrange(
                    "s (i j) -> (s i) j", j=DM) for t in (o1a, o1b)]

            def emit_stage(si, w):
                # ---- input (fwd tokens only) ----
                in1 = work.tile([128, NT], fp16, tag="in1")
                in2 = work.tile([64, NT], fp16, tag="in2")
                if si == 0:
                    # scalar queue is DMA-free at t=0; these must land first
                    nc.scalar.dma_start(in1[:], x1_d[:])
                    nc.scalar.dma_start(in2[:], x2_d[:])
                else:
                    for t in range(7):
                        x2f = sw.tile([112, DM], fp16, tag="x2f")
                        # readbacks on the gpsimd queue: the sync queue is
                        # busy issuing the o1 writes at this point
                        r0, r1 = t * 112, (t + 1) * 112
                        if r1 <= 392:
                            nc.gpsimd.dma_start(x2f[:], o1f[0][r0:r1, :])
                        elif r0 >= 392:
                            nc.gpsimd.dma_start(x2f[:], o1f[1][r0 - 392:r1 - 392, :])
                        else:
                            nc.gpsimd.dma_start(x2f[0:392 - r0, :],
                                                o1f[0][r0:392, :])
                            nc.gpsimd.dma_start(x2f[392 - r0:112, :],
                                                o1f[1][0:r1 - 392, :])
                        pt1 = pmm.tile([128, 112], fp16, tag="mm")
                        nc.tensor.matmul(pt1[:], x2f[:, 0:128],
                                         identsb[0:112, 0:112],
                                         is_transpose=True, start=True, stop=True)
                        nc.vector.tensor_scalar_add(
                            in1[:, t * 112:(t + 1) * 112], pt1[:], 0.0)
                        pt2 = pmm.tile([64, 112], fp16, tag="mm")
                        nc.tensor.matmul(pt2[:], x2f[:, 128:192],
                                         identsb[0:112, 0:112],
                                         is_transpose=True, start=True, stop=True)
                        nc.vector.tensor_scalar_add(
                            in2[:, t * 112:(t + 1) * 112], pt2[:], 0.0)

                # ---- in_proj helpers: the xi part (e<3) is head-critical,
                # the z part (silu gate, needed only at gating) is deferred
                # into the jc0 n-loop ----
                siluz = []
                for e in range(3, 6):
                    t = work.tile([128, NT], fp16, tag=f"siluz{e - 3}")
                    siluz.append(t)

                def in_proj_unit(e, j, pool, ptag):
                    t0 = j * TC
                    pm = pool.tile([128, TC], f32, tag=ptag, name="pm")
                    nc.tensor.matmul(pm[:], w["inw1"][:, e * 128:(e + 1) * 128],
                                     in1[:, t0:t0 + TC], start=True, stop=False)
                    nc.tensor.matmul(pm[:], w["inw2"][:, e * 128:(e + 1) * 128],
                                     in2[:, t0:t0 + TC], start=False, stop=True)
                    if e < 3:
                        # copy on DVE - it idles in phase A
                        r3 = xi_pad[("f", e)][:].rearrange(
                            "p (s q) -> p s q", q=PADS)
                        nc.vector.tensor_scalar_add(
                            r3[:, 7 * j:7 * j + 7, 2:PADS],
                            pm[:].rearrange("p (s q) -> p s q", q=L), 0.0)
                    else:
                        nc.scalar.activation(siluz[e - 3][:, t0:t0 + TC],
                                             pm[:], Act.Silu)

                def rev_xi(ct):
                    rf = xi_pad[("f", ct)][:].rearrange("p (s q) -> p s q", q=PADS)
                    rr = xi_pad[("r", ct)][:].rearrange("p (s q) -> p s q", q=PADS)
                    src = rf[:, :, 2:PADS][:, :, ::-1]
                    nc.vector.tensor_tensor(rr[:, :, 2:PADS], src, src, Alu.bypass)

                # ---- conv / x_proj / dt helpers; fwd half emitted up-front,
                # rev half interleaved into the jc0 n-loop so its PE/Scalar
                # work hides under the DVE-bound scans (PE queues are
                # in-order, so emission order = PE execution order) ----
                xcb, dtsp, dtxc = [], [], []
                for ct in range(3):
                    xc = work.tile([128, T2], fp16, tag=f"xcb{ct}")
                    xcb.append(xc)
                    t = work.tile([128, T2], fp16, tag=f"dtsp{ct}")
                    dtsp.append(t)
                    tx = work.tile([128, T2], fp16, tag=f"dtxc{ct}")
                    dtxc.append(tx)
                dbl = work.tile([DR + 2 * DS, T2], fp16, tag="dbl")
                tsp32 = w1.tile([128, 3 * T2], f32, tag="tsp32")

                def conv_group(di, j, ct, pool=None, ptag=None):
                    # rev half (di=1) runs inside the jc0 n-loop while all 6
                    # "acc" banks are held -> must draw from the "mm" ring
                    if pool is None:
                        pool, ptag = (paccp, "acc") if di == 0 else (pmm, "mm")
                    dr = "fr"[di]
                    r3 = xi_pad[(dr, ct)][:].rearrange("p (s q) -> p s q", q=PADS)
                    pm = pool.tile([128, TC], f32, tag=ptag, name="pm")
                    pm3 = pm[:].rearrange("p (s l) -> p s l", l=L)
                    for k in range(3):
                        nc.tensor.matmul(
                            pm3, w[f"cdiag{ct}"][:, k * 128:(k + 1) * 128],
                            r3[:, 7 * j:7 * j + 7, k:k + L],
                            start=(k == 0), stop=(k == 2))
                    nc.scalar.activation(
                        xcb[ct][:, di * NT + j * TC:di * NT + (j + 1) * TC],
                        pm[:], Act.Silu, bias=w[f"convb{ct}"][:, 0:1])

                def xproj(j, pool, ptag):
                    t0 = j * TC
                    pd = pool.tile([DR + 2 * DS, TC], f32, tag=ptag, name="pd")
                    for ct in range(3):
                        nc.tensor.matmul(pd[:], w[f"xpw{ct}"][:],
                                         xcb[ct][:, t0:t0 + TC],
                                         start=(ct == 0), stop=(ct == 2))
                    nc.vector.tensor_scalar_add(dbl[:, t0:t0 + TC], pd[:], 0.0)

                def dt_unit(j, ct, pool, ptag):
                    t0 = j * TC
                    pm = pool.tile([128, TC], f32, tag=ptag, name="pm")
                    nc.tensor.matmul(pm[:], w["dtw"][:, ct * 128:(ct + 1) * 128],
                                     dbl[0:DR, t0:t0 + TC], start=True, stop=True)
                    nc.scalar.activation(tsp32[:, ct * T2 + t0:ct * T2 + t0 + TC],
                                         pm[:], Act.Exp,
                                         bias=w[f"dtb{ct}"][:, 0:1])
                    nc.scalar.activation(dtsp[ct][:, t0:t0 + TC],
                                         tsp32[:, ct * T2 + t0:ct * T2 + t0 + TC],
                                         Act.Ln, bias=1.0)

                def xproj_dt(j):
                    xproj(j, paccp, "acc")
                    for ct in range(3):
                        dt_unit(j, ct, paccp, "acc")

                def dtxc_ct(jc, ct):
                    jO = jc * NT
                    nc.vector.tensor_tensor(dtxc[ct][:, jO:jO + NT],
                                            dtsp[ct][:, jO:jO + NT],
                                            xcb[ct][:, jO:jO + NT], Alu.mult)
                    r3 = dtsp[ct][:, jO:jO + NT].rearrange(
                        "p (s l) -> p s l", l=L)
                    nc.vector.memset(r3[:, :, 0:1], POISON)

                def dtxc_half(jc):
                    for ct in range(3):
                        dtxc_ct(jc, ct)

                # head: only what the jc0 loop needs - in_proj-xi + conv-fwd
                # + x_proj/dt fwd chunks. z-projection and the whole rev half
                # are deferred into the jc0 n-loop.
                for j in range(2):
                    for e in range(3):
                        in_proj_unit(e, j, paccp, "acc")
                    for ct in range(3):
                        conv_group(0, j, ct)
                for ct in range(3):
                    rev_xi(ct)
                for j in range(2):
                    xproj_dt(j)
                dtxc_half(0)

                # deferred work units, emitted inside the jc0 n-loop (after
                # the ct0 block so they don't delay dA0). n=2..7 carry one
                # rev-conv + one z-projection unit; n=8..15 carry the rev
                # x_proj / dt chain so the jc0->jc1 transition is short.
                sched = {}
                for i, (jj, ct) in enumerate([(j, ct) for j in range(2)
                                              for ct in range(3)]):
                    sched.setdefault(2 + i, []).append(
                        lambda j=jj, c=ct: conv_group(1, j, c))
                for i, (jj, e) in enumerate([(j, e) for j in range(2)
                                             for e in range(3, 6)]):
                    sched.setdefault(2 + i, []).append(
                        lambda e=e, j=jj: in_proj_unit(e, j, pmm, "mm"))
                sched[8] = [lambda: xproj(2, pmm, "mm")]
                sched[9] = [lambda: dt_unit(2, 0, pmm, "mm")]
                sched[10] = [lambda: dt_unit(2, 1, pmm, "mm")]
                sched[11] = [lambda: dt_unit(2, 2, pmm, "mm")]
                sched[12] = [lambda: xproj(3, pmm, "mm")]
                sched[13] = [lambda: dt_unit(3, 0, pmm, "mm")]
                sched[14] = [lambda: dt_unit(3, 1, pmm, "mm"),
                             lambda: dtxc_ct(1, 0)]
                sched[15] = [lambda: dt_unit(3, 2, pmm, "mm"),
                             lambda: dtxc_ct(1, 1)]

                # ---- scan loop: two 784-token halves (fwd, rev) ----
                gsum, grs = [], []
                for ct in range(3):
                    g = work.tile([128, NT], fp16, tag=f"gs{ct}")
                    gsum.append(g)
                for jc in range(2):
                    jO = jc * NT
                    # 6 held acc banks, seeded with diag(D) @ xc
                    acc = {}
                    for ct in range(3):
                        for q in range(2):
                            a = paccp.tile([128, TC], f32, tag="acc", name="a")
                            nc.tensor.matmul(
                                a[:], w[f"ddiag{ct}"][:],
                                xcb[ct][:, jO + q * TC:jO + (q + 1) * TC],
                                start=True, stop=False)
                            acc[(ct, q)] = a
                    for n in range(DS):
                        pc_sb = bcp.tile([128, NT], fp16, tag="pc")
                        pb_sb = bcp.tile([128, NT], fp16, tag="pb")
                        for q in range(2):
                            t0 = jO + q * TC
                            pb = pmm.tile([128, TC], f32, tag="mm")
                            nc.tensor.matmul(pb[:], selC[:, n * 128:(n + 1) * 128],
                                             dbl[:, t0:t0 + TC],
                                             start=True, stop=True)
                            nc.scalar.copy(pc_sb[:, q * TC:(q + 1) * TC], pb[:])
                        for q in range(2):
                            t0 = jO + q * TC
                            pb = pmm.tile([128, TC], f32, tag="mm")
                            nc.tensor.matmul(pb[:], selB[:, n * 128:(n + 1) * 128],
                                             dbl[:, t0:t0 + TC],
                                             start=True, stop=True)
                            nc.scalar.copy(pb_sb[:, q * TC:(q + 1) * TC], pb[:])
                        for ct in range(3):
                            dA = sw.tile([128, NT], fp16, tag=f"dA{ct}")
                            nc.scalar.activation(dA[:], dtsp[ct][:, jO:jO + NT],
                                                 Act.Exp,
                                                 scale=w[f"A{ct}"][:, n:n + 1])
                            dBx = sw.tile([128, NT], fp16, tag=f"dBx{ct}")
                            nc.vector.tensor_tensor(dBx[:], dtxc[ct][:, jO:jO + NT],
                                                    pb_sb[:], Alu.mult)
                            hs = sw.tile([128, NT], fp16, tag=f"hs{ct}")
                            nc.vector.tensor_tensor_scan(hs[:], dA[:], dBx[:],
                                                         0.0, Alu.mult, Alu.add)
                            yt = sw.tile([128, NT], fp16, tag=f"yt{ct}")
                            nc.vector.tensor_tensor(yt[:], hs[:], pc_sb[:], Alu.mult)
                            for q in range(2):
                                nc.tensor.matmul(acc[(ct, q)][:], identsb[:],
                                                 yt[:, q * TC:(q + 1) * TC],
                                                 start=False, stop=(n == DS - 1))
                            if ct == 0 and jc == 0:
                                for f in sched.get(n, ()):
                                    f()
                    # ---- gate this half with silu(z). acc banks are copied
                    # to SBUF fp16 by Scalar first so the DVE gate multiplies
                    # run in 2x packed mode instead of the 1x PSUM mode ----
                    for ct in range(3):
                        ya = sw.tile([128, NT], fp16, tag=f"ya{ct}")
                        for q in range(2):
                            nc.scalar.copy(ya[:, q * TC:(q + 1) * TC],
                                           acc[(ct, q)][:])
                        sz3 = siluz[ct][:].rearrange("p (s l) -> p s l", l=L)
                        if jc == 0:
                            nc.vector.tensor_tensor(gsum[ct][:], siluz[ct][:],
                                                    ya[:], Alu.mult)
                        else:
                            gr = sw.tile([128, NT], fp16, tag=f"gr{ct}")
                            gr3 = gr[:].rearrange("p (s l) -> p s l", l=L)
                            nc.vector.tensor_tensor(
                                gr3[:, :, :], sz3[:, :, ::-1],
                                ya[:].rearrange("p (s l) -> p s l", l=L),
                                Alu.mult)
                            grs.append(gr)
                    if jc == 0:
                        dtxc_ct(1, 2)

                # ---- bidirectional sum + out_proj, chunked so the first
                # out chunk (and stage-2's readback) starts asap ----
                for j in range(2):
                    t0 = j * TC
                    for ct in range(3):
                        nc.vector.tensor_tensor(gsum[ct][:, t0:t0 + TC],
                                                gsum[ct][:, t0:t0 + TC],
                                                grs[ct][:, t0:t0 + TC], Alu.add)
                    for dchunk, dlo, dhi in ((0, 0, 128), (1, 128, 192)):
                        po = pmm.tile([dhi - dlo, TC], f32, tag="mm")
                        for ct in range(3):
                            nc.tensor.matmul(po[:], w[f"outw{ct}"][:, dlo:dhi],
                                             gsum[ct][:, t0:t0 + TC],
                                             start=(ct == 0), stop=(ct == 2))
                        if si == 0:
                            osb = w1.tile([dhi - dlo, TC], fp16, tag=f"osb{dchunk}")
                            nc.scalar.copy(osb[:], po[:])
                            o1x = o1a if j == 0 else o1b
                            dap = o1x[:, dlo:dhi, :].transpose([1, 0, 2])
                            nc.sync.dma_start(
                                dap[:, :, :],
                                osb[:].rearrange("p (s l) -> p s l", l=L))
                        else:
                            osb = w1.tile([dhi - dlo, TC], f32, tag=f"osbf{dchunk}")
                            nc.scalar.copy(osb[:], po[:])
                            nc.sync.dma_start(out_d[dlo:dhi, t0:t0 + TC], osb[:])

            emit_stage(0, wts["h"])
            emit_stage(1, wts["w"])

    split_excess_waits(nc)
    return nc


_NC_CACHE = None


def _get_nc():
    global _NC_CACHE
    if _NC_CACHE is None:
        _NC_CACHE = build_nc()
    return _NC_CACHE


def build_in_maps(inputs):
    inputs = {k: np.asarray(v, dtype=np.float32) for k, v in inputs.items()}
    x = inputs["x"]
    h16 = lambda a: np.ascontiguousarray(a.astype(np.float16))

    selB = np.zeros((DR + 2 * DS, DS * 128), np.float16)
    selC = np.zeros((DR + 2 * DS, DS * 128), np.float16)
    for n in range(DS):
        selB[DR + n, n * 128:(n + 1) * 128] = 1.0
        selC[DR + DS + n, n * 128:(n + 1) * 128] = 1.0
    base = {"ident": np.eye(128, dtype=np.float16),
            "selB": selB, "selC": selC}
    for p, tag in (("h", "h_"), ("w", "w_")):
        inw = inputs[tag + "in_proj_w"].T                        # [192, 768]
        base[f"{p}_inw1"] = h16(inw[0:128, :])
        base[f"{p}_inw2"] = h16(inw[128:192, :])
        base[f"{p}_xpw"] = h16(inputs[tag + "x_proj_w"].T)
        base[f"{p}_dtw"] = h16(inputs[tag + "dt_proj_w"].T)
        base[f"{p}_outw"] = h16(inputs[tag + "out_proj_w"].T)
        cw = inputs[tag + "conv_w"]                              # [384, 3]
        cdiag = np.zeros((DI, 3 * 128), np.float16)
        for ct in range(3):
            for k in range(3):
                cdiag[ct * 128:(ct + 1) * 128, k * 128:(k + 1) * 128] = \
                    np.diag(cw[ct * 128:(ct + 1) * 128, k].astype(np.float16))
        base[f"{p}_cdiag"] = cdiag
        dsk = inputs[tag + "D_skip"]
        ddiag = np.zeros((DI, 128), np.float16)
        for ct in range(3):
            ddiag[ct * 128:(ct + 1) * 128, :] = \
                np.diag(dsk[ct * 128:(ct + 1) * 128].astype(np.float16))
        base[f"{p}_ddiag"] = ddiag
        base[f"{p}_convb"] = inputs[tag + "conv_b"].reshape(DI, 1).copy()
        base[f"{p}_dtb"] = inputs[tag + "dt_proj_b"].reshape(DI, 1).copy()
        base[f"{p}_A"] = np.ascontiguousarray(-np.exp(inputs[tag + "A_log"]))

    in_maps = []
    for core in range(NCORE):
        sl = range(core * SPC, (core + 1) * SPC)
        seqs = np.stack([x[s // W, :, s % W, :] for s in sl])     # [14, 56, 192]
        xt = np.ascontiguousarray(seqs.reshape(NT, DM).T)         # [192, 784]
        m = dict(base)
        m["x1"] = h16(xt[0:128, :])
        m["x2"] = h16(xt[128:192, :])
        in_maps.append(m)
    return in_maps


def kernel(**inputs):
    in_maps = build_in_maps(inputs)
    nc = _get_nc()
    res = run_bass_kernel_spmd(nc, in_maps, core_ids=list(range(NCORE)))

    out_full = np.zeros((NSEQ, L, DM), np.float32)
    for core in range(NCORE):
        o = np.asarray(res.results[core]["out"], dtype=np.float32)   # [192, 784]
        out_full[core * SPC:(core + 1) * SPC] = o.T.reshape(SPC, L, DM)
    return out_full.reshape(B, H, W, DM)


# revision 5
# speedup vs baseline: 1.0376x; 1.0008x over previous
"""BiMamba2DFast kernel for 8 Trainium2 NeuronCores (Bass/Tile), v3.

Data-parallel over the 112 (b, w)-sequences - 14 per core. Key changes vs v2:
GpSimd does no elementwise work (its TT ops share the DVE SBUF port and
roughly double both engines' per-op time - measured). The scan loop is
restructured into two 784-token halves (fwd stream, rev stream) so that all
THREE 128-channel blocks accumulate y over n on the PE into held PSUM banks
(6 acc banks + 2 broadcast banks = 8). B rows are cast to fp16 SBUF like C
so the dBx/yt multiplies run in the DVE 2x packed mode. The scan is fp16
tensor_tensor_scan (fp32 internal state), segmented by poisoning dt at
sequence starts.
"""
import sys

sys.path.insert(0, '/opt/trn_rl_repo')

import numpy as np
import concourse.bass as bass
import concourse.tile as tile
from concourse import mybir
from concourse.bass_utils import run_bass_kernel_spmd
import bass_rust

f32 = mybir.dt.float32
fp16 = mybir.dt.float16
Alu = mybir.AluOpType
Act = mybir.ActivationFunctionType

DM, DI, DS, DR = 192, 384, 16, 12
B, H, W, L = 2, 56, 56, 56
NSEQ = B * W                 # 112
NCORE = 8
SPC = NSEQ // NCORE          # 14 sequences per core
NT = SPC * L                 # 784 fwd tokens
T2 = 2 * NT                  # 1568 fwd+rev tokens
TC = 392
PADS = L + 2                 # 58 cols per seq in conv buffer
POISON = 30000.0

_nop_ctr = [0]


def _make_wait_nop(engine, wait):
    _nop_ctr[0] += 1
    inst = bass_rust.InstNoOp(name=f"waitnop-{_nop_ctr[0]}", hint="splitwait",
                              cycle_cnt=0)
    inst.engine = engine
    inst.sync_info = bass_rust.SyncInfo(on_wait=[wait], on_update=[])
    return inst


def split_excess_waits(nc, max_waits=1):
    """This walrus build rejects >max_waits sem waits per instruction; hoist
    the excess onto same-engine NoOps placed just before the instruction."""
    for fn in nc.m.functions:
        for bb in fn.blocks:
            if not any(inst.sync_info is not None and inst.sync_info.on_wait
                       and len(inst.sync_info.on_wait) > max_waits
                       for inst in bb.instructions):
                continue
            new_list = []
            for inst in bb.instructions:
                si = inst.sync_info
                if si is not None and si.on_wait and len(si.on_wait) > max_waits:
                    waits = list(si.on_wait)
                    keep = waits[-max_waits:]
                    for w in waits[:-max_waits]:
                        new_list.append(_make_wait_nop(inst.engine, w))
                    si.on_wait = keep
                new_list.append(inst)
            bb.instructions[:] = new_list


def build_nc():
    nc = bass.Bass()

    def din(nm, sh, dt=fp16):
        return nc.declare_dram_parameter(nm, list(sh), dt, isOutput=False)

    x1_d = din("x1", (128, NT))
    x2_d = din("x2", (64, NT))
    ident_d = din("ident", (128, 128))
    selB_d = din("selB", (DR + 2 * DS, DS * 128))
    selC_d = din("selC", (DR + 2 * DS, DS * 128))
    wt_dram = {}
    for p in ("h", "w"):
        wt_dram[p] = dict(
            inw1=din(f"{p}_inw1", (128, 2 * DI)),
            inw2=din(f"{p}_inw2", (64, 2 * DI)),
            xpw=din(f"{p}_xpw", (DI, DR + 2 * DS)),
            dtw=din(f"{p}_dtw", (DR, DI)),
            outw=din(f"{p}_outw", (DI, DM)),
            cdiag=din(f"{p}_cdiag", (DI, 3 * 128)),      # diag(conv_w[:,k]) blocks
            ddiag=din(f"{p}_ddiag", (DI, 128)),          # diag(D_skip)
            convb=din(f"{p}_convb", (DI, 1), f32),
            dtb=din(f"{p}_dtb", (DI, 1), f32),
            A=din(f"{p}_A", (DI, DS), f32),
        )
    out_d = nc.declare_dram_parameter("out", [DM, NT], f32, isOutput=True)

    with tile.TileContext(nc) as tc:
        with (
            tc.tile_pool(name="pers", bufs=1) as pers,
            tc.tile_pool(name="work", bufs=1) as work,
            tc.tile_pool(name="sw", bufs=3) as sw,
            tc.tile_pool(name="w1", bufs=1) as w1,
            tc.tile_pool(name="bc", bufs=2) as bcp,
            tc.tile_pool(name="mm", bufs=2, space=bass.MemorySpace.PSUM) as pmm,
            tc.tile_pool(name="acc", bufs=6, space=bass.MemorySpace.PSUM) as paccp,
            tc.tile_pool(name="dram", bufs=1, space="DRAM") as dpool,
        ):
            # o1 split by 7-seq halves so stage 2's readback of the first
            # sequences can start while stage 1 still writes the second half
            o1a = dpool.tile([SPC // 2, DM, L], fp16, tag="o1a")
            o1b = dpool.tile([SPC // 2, DM, L], fp16, tag="o1b")

            # weight-load DMAs: issue order matters (~650ns of issue time
            # each, serialized per queue). Load stage-h weights in pipeline
            # order on gpsimd+sync (scalar stays free for phase-A ACT work);
            # stage-w weights go last - they aren't needed for ~300us.
            _ectr = [0]

            def wload(dst, src):
                e = nc.gpsimd if _ectr[0] % 2 == 0 else nc.sync
                _ectr[0] += 1
                e.dma_start(dst, src)

            wts = {}
            load2 = []
            for p in ("h", "w"):
                d = wt_dram[p]
                w = {}
                for nm in ("inw1", "inw2", "dtw"):
                    t = pers.tile(list(d[nm].shape), fp16, tag=f"{p}{nm}")
                    if p == "h" and nm != "dtw":
                        wload(t[:], d[nm][:])
                    else:
                        load2.append((t[:], d[nm][:]))
                    w[nm] = t
                for ct in range(3):
                    for nm, cols, dt in (("cdiag", 3 * 128, fp16),
                                         ("convb", 1, f32),
                                         ("xpw", DR + 2 * DS, fp16),
                                         ("dtb", 1, f32), ("A", DS, f32),
                                         ("ddiag", 128, fp16),
                                         ("outw", DM, fp16)):
                        t = pers.tile([128, cols], dt, tag=f"{p}{nm}{ct}")
                        if p == "h" and nm in ("cdiag", "convb"):
                            wload(t[:], d[nm][ct * 128:(ct + 1) * 128, :])
                        else:
                            load2.append((t[:], d[nm][ct * 128:(ct + 1) * 128, :]))
                        w[f"{nm}{ct}"] = t
                wts[p] = w

            identsb = pers.tile([128, 128], fp16, tag="ident")
            wload(identsb[:], ident_d[:])
            selB = pers.tile([DR + 2 * DS, DS * 128], fp16, tag="selB")
            wload(selB[:], selB_d[:])
            selC = pers.tile([DR + 2 * DS, DS * 128], fp16, tag="selC")
            wload(selC[:], selC_d[:])
            for dst, src in load2:
                wload(dst, src)

            # conv padded buffers [128, 14, 58]; zero the 2 pad cols once
            xi_pad = {}
            for dr in ("f", "r"):
                for ct in range(3):
                    t = pers.tile([128, SPC * PADS], fp16, tag=f"xp{dr}{ct}")
                    r3 = t[:].rearrange("p (s q) -> p s q", q=PADS)
                    nc.vector.memset(r3[:, :, 0:2], 0.0)
                    xi_pad[(dr, ct)] = t

            o1f = [
                t[:].rearrange("s d h -> s (d h)").rearrange(
                    "s (i j) -> (s i) j", j=DM) for t in (o1a, o1b)]

            def emit_stage(si, w):
                # ---- input (fwd tokens only) ----
                in1 = work.tile([128, NT], fp16, tag="in1")
                in2 = work.tile([64, NT], fp16, tag="in2")
                if si == 0:
                    # scalar queue is DMA-free at t=0; these must land first
                    nc.scalar.dma_start(in1[:], x1_d[:])
                    nc.scalar.dma_start(in2[:], x2_d[:])
                else:
                    for t in range(7):
                        x2f = sw.tile([112, DM], fp16, tag="x2f")
                        # readbacks on the gpsimd queue: the sync queue is
                        # busy issuing the o1 writes at this point
                        r0, r1 = t * 112, (t + 1) * 112
                        if r1 <= 392:
                            nc.gpsimd.dma_start(x2f[:], o1f[0][r0:r1, :])
                        elif r0 >= 392:
                            nc.gpsimd.dma_start(x2f[:], o1f[1][r0 - 392:r1 - 392, :])
                        else:
                            nc.gpsimd.dma_start(x2f[0:392 - r0, :],
                                                o1f[0][r0:392, :])
                            nc.gpsimd.dma_start(x2f[392 - r0:112, :],
                                                o1f[1][0:r1 - 392, :])
                        pt1 = pmm.tile([128, 112], fp16, tag="mm")
                        nc.tensor.matmul(pt1[:], x2f[:, 0:128],
                                         identsb[0:112, 0:112],
                                         is_transpose=True, start=True, stop=True)
                        nc.vector.tensor_scalar_add(
                            in1[:, t * 112:(t + 1) * 112], pt1[:], 0.0)
                        pt2 = pmm.tile([64, 112], fp16, tag="mm")
                        nc.tensor.matmul(pt2[:], x2f[:, 128:192],
                                         identsb[0:112, 0:112],
                                         is_transpose=True, start=True, stop=True)
                        nc.vector.tensor_scalar_add(
                            in2[:, t * 112:(t + 1) * 112], pt2[:], 0.0)

                # ---- in_proj helpers: the xi part (e<3) is head-critical,
                # the z part (silu gate, needed only at gating) is deferred
                # into the jc0 n-loop ----
                siluz = []
                for e in range(3, 6):
                    t = work.tile([128, NT], fp16, tag=f"siluz{e - 3}")
                    siluz.append(t)

                def in_proj_unit(e, j, pool, ptag):
                    t0 = j * TC
                    pm = pool.tile([128, TC], f32, tag=ptag, name="pm")
                    nc.tensor.matmul(pm[:], w["inw1"][:, e * 128:(e + 1) * 128],
                                     in1[:, t0:t0 + TC], start=True, stop=False)
                    nc.tensor.matmul(pm[:], w["inw2"][:, e * 128:(e + 1) * 128],
                                     in2[:, t0:t0 + TC], start=False, stop=True)
                    if e < 3:
                        # copy on DVE - it idles in phase A
                        r3 = xi_pad[("f", e)][:].rearrange(
                            "p (s q) -> p s q", q=PADS)
                        nc.vector.tensor_scalar_add(
                            r3[:, 7 * j:7 * j + 7, 2:PADS],
                            pm[:].rearrange("p (s q) -> p s q", q=L), 0.0)
                    else:
                        # z units run inside the n-loop: stage pre-activation
                        # (copy has no table cost), silu_batch applies silu
                        nc.scalar.copy(
                            zstage[:, (j * 3 + e - 3) * TC:(j * 3 + e - 2) * TC],
                            pm[:])

                def rev_xi(ct):
                    rf = xi_pad[("f", ct)][:].rearrange("p (s q) -> p s q", q=PADS)
                    rr = xi_pad[("r", ct)][:].rearrange("p (s q) -> p s q", q=PADS)
                    src = rf[:, :, 2:PADS][:, :, ::-1]
                    nc.vector.tensor_tensor(rr[:, :, 2:PADS], src, src, Alu.bypass)

                # ---- conv / x_proj / dt helpers; fwd half emitted up-front,
                # rev half interleaved into the jc0 n-loop so its PE/Scalar
                # work hides under the DVE-bound scans (PE queues are
                # in-order, so emission order = PE execution order) ----
                xcb, dtsp, dtxc = [], [], []
                for ct in range(3):
                    xc = work.tile([128, T2], fp16, tag=f"xcb{ct}")
                    xcb.append(xc)
                    t = work.tile([128, T2], fp16, tag=f"dtsp{ct}")
                    dtsp.append(t)
                    tx = work.tile([128, T2], fp16, tag=f"dtxc{ct}")
                    dtxc.append(tx)
                dbl = work.tile([DR + 2 * DS, T2], fp16, tag="dbl")
                tsp32 = w1.tile([128, 3 * T2], f32, tag="tsp32")
                # pre-activation staging for the deferred (batched) silus
                cstage = work.tile([128, 6 * TC], fp16, tag="cstage")
                zstage = work.tile([128, 6 * TC], fp16, tag="zstage")

                def conv_group(di, j, ct, pool=None, ptag=None):
                    # rev half (di=1) runs inside the jc0 n-loop while all 6
                    # "acc" banks are held -> must draw from the "mm" ring.
                    # In-loop silus would thrash the ACT table set (silu<->exp
                    # around every dA, ~2.6us per iteration), so di=1 only
                    # COPIES the pre-activation to staging; silu_batch() below
                    # applies silu in two table-switch-free groups.
                    if pool is None:
                        pool, ptag = (paccp, "acc") if di == 0 else (pmm, "mm")
                    dr = "fr"[di]
                    r3 = xi_pad[(dr, ct)][:].rearrange("p (s q) -> p s q", q=PADS)
                    pm = pool.tile([128, TC], f32, tag=ptag, name="pm")
                    pm3 = pm[:].rearrange("p (s l) -> p s l", l=L)
                    for k in range(3):
                        nc.tensor.matmul(
                            pm3, w[f"cdiag{ct}"][:, k * 128:(k + 1) * 128],
                            r3[:, 7 * j:7 * j + 7, k:k + L],
                            start=(k == 0), stop=(k == 2))
                    if di == 1:
                        nc.scalar.copy(
                            cstage[:, (j * 3 + ct) * TC:(j * 3 + ct + 1) * TC],
                            pm[:])
                    else:
                        nc.scalar.activation(
                            xcb[ct][:, di * NT + j * TC:di * NT + (j + 1) * TC],
                            pm[:], Act.Silu, bias=w[f"convb{ct}"][:, 0:1])

                def silu_batch(j):
                    # one silu-set table residency covers all 6 activations
                    for ct in range(3):
                        nc.scalar.activation(
                            xcb[ct][:, NT + j * TC:NT + (j + 1) * TC],
                            cstage[:, (j * 3 + ct) * TC:(j * 3 + ct + 1) * TC],
                            Act.Silu, bias=w[f"convb{ct}"][:, 0:1])
                    for e in range(3):
                        nc.scalar.activation(
                            siluz[e][:, j * TC:(j + 1) * TC],
                            zstage[:, (j * 3 + e) * TC:(j * 3 + e + 1) * TC],
                            Act.Silu)

                def xproj(j, pool, ptag):
                    t0 = j * TC
                    pd = pool.tile([DR + 2 * DS, TC], f32, tag=ptag, name="pd")
                    for ct in range(3):
                        nc.tensor.matmul(pd[:], w[f"xpw{ct}"][:],
                                         xcb[ct][:, t0:t0 + TC],
                                         start=(ct == 0), stop=(ct == 2))
                    nc.vector.tensor_scalar_add(dbl[:, t0:t0 + TC], pd[:], 0.0)

                def dt_unit(j, ct, pool, ptag):
                    t0 = j * TC
                    pm = pool.tile([128, TC], f32, tag=ptag, name="pm")
                    nc.tensor.matmul(pm[:], w["dtw"][:, ct * 128:(ct + 1) * 128],
                                     dbl[0:DR, t0:t0 + TC], start=True, stop=True)
                    nc.scalar.activation(tsp32[:, ct * T2 + t0:ct * T2 + t0 + TC],
                                         pm[:], Act.Exp,
                                         bias=w[f"dtb{ct}"][:, 0:1])
                    nc.scalar.activation(dtsp[ct][:, t0:t0 + TC],
                                         tsp32[:, ct * T2 + t0:ct * T2 + t0 + TC],
                                         Act.Ln, bias=1.0)

                def xproj_dt(j):
                    xproj(j, paccp, "acc")
                    for ct in range(3):
                        dt_unit(j, ct, paccp, "acc")

                def dtxc_ct(jc, ct):
                    jO = jc * NT
                    nc.vector.tensor_tensor(dtxc[ct][:, jO:jO + NT],
                                            dtsp[ct][:, jO:jO + NT],
                                            xcb[ct][:, jO:jO + NT], Alu.mult)
                    r3 = dtsp[ct][:, jO:jO + NT].rearrange(
                        "p (s l) -> p s l", l=L)
                    nc.vector.memset(r3[:, :, 0:1], POISON)

                def dtxc_half(jc):
                    for ct in range(3):
                        dtxc_ct(jc, ct)

                # head: only what the jc0 loop needs - in_proj-xi + conv-fwd
                # + x_proj/dt fwd chunks. z-projection and the whole rev half
                # are deferred into the jc0 n-loop.
                for j in range(2):
                    for e in range(3):
                        in_proj_unit(e, j, paccp, "acc")
                    for ct in range(3):
                        conv_group(0, j, ct)
                for ct in range(3):
                    rev_xi(ct)
                for j in range(2):
                    xproj_dt(j)
                dtxc_half(0)

                # deferred work units, emitted inside the jc0 n-loop (after
                # the ct0 block). conv/z units only stage pre-activations;
                # silu_batch(j) applies all 6 silus of a j-half in one
                # table-set residency, placed just before its x_proj consumer.
                z = lambda e, j: in_proj_unit(e, j, pmm, "mm")
                cv = lambda j, c: conv_group(1, j, c)
                sched = {
                    2: [lambda: cv(0, 0)],
                    3: [lambda: cv(0, 1), lambda: z(3, 0)],
                    4: [lambda: cv(0, 2), lambda: z(4, 0)],
                    5: [lambda: z(5, 0), lambda: silu_batch(0)],
                    6: [lambda: xproj(2, pmm, "mm")],
                    7: [lambda: cv(1, 0), lambda: dt_unit(2, 0, pmm, "mm")],
                    8: [lambda: cv(1, 1), lambda: dt_unit(2, 1, pmm, "mm")],
                    9: [lambda: cv(1, 2), lambda: dt_unit(2, 2, pmm, "mm")],
                    10: [lambda: z(3, 1), lambda: z(4, 1)],
                    11: [lambda: z(5, 1), lambda: silu_batch(1)],
                    12: [lambda: xproj(3, pmm, "mm")],
                    13: [lambda: dt_unit(3, 0, pmm, "mm"),
                         lambda: dtxc_ct(1, 0)],
                    14: [lambda: dt_unit(3, 1, pmm, "mm"),
                         lambda: dtxc_ct(1, 1)],
                    15: [lambda: dt_unit(3, 2, pmm, "mm"),
                         lambda: dtxc_ct(1, 2)],
                }

                # ---- scan loop: two 784-token halves (fwd, rev) ----
                gsum, grs = [], []
                for ct in range(3):
                    g = work.tile([128, NT], fp16, tag=f"gs{ct}")
                    gsum.append(g)
                for jc in range(2):
                    jO = jc * NT
                    # 6 held acc banks, seeded with diag(D) @ xc
                    acc = {}
                    for ct in range(3):
                        for q in range(2):
                            a = paccp.tile([128, TC], f32, tag="acc", name="a")
                            nc.tensor.matmul(
                                a[:], w[f"ddiag{ct}"][:],
                                xcb[ct][:, jO + q * TC:jO + (q + 1) * TC],
                                start=True, stop=False)
                            acc[(ct, q)] = a
                    for n in range(DS):
                        pc_sb = bcp.tile([128, NT], fp16, tag="pc")
                        pb_sb = bcp.tile([128, NT], fp16, tag="pb")
                        for q in range(2):
                            t0 = jO + q * TC
                            pb = pmm.tile([128, TC], f32, tag="mm")
                            nc.tensor.matmul(pb[:], selC[:, n * 128:(n + 1) * 128],
                                             dbl[:, t0:t0 + TC],
                                             start=True, stop=True)
                            nc.scalar.copy(pc_sb[:, q * TC:(q + 1) * TC], pb[:])
                        for q in range(2):
                            t0 = jO + q * TC
                            pb = pmm.tile([128, TC], f32, tag="mm")
                            nc.tensor.matmul(pb[:], selB[:, n * 128:(n + 1) * 128],
                                             dbl[:, t0:t0 + TC],
                                             start=True, stop=True)
                            nc.scalar.copy(pb_sb[:, q * TC:(q + 1) * TC], pb[:])
                        for ct in range(3):
                            dA = sw.tile([128, NT], fp16, tag=f"dA{ct}")
                            nc.scalar.activation(dA[:], dtsp[ct][:, jO:jO + NT],
                                                 Act.Exp,
                                                 scale=w[f"A{ct}"][:, n:n + 1])
                            dBx = sw.tile([128, NT], fp16, tag=f"dBx{ct}")
                            nc.vector.tensor_tensor(dBx[:], dtxc[ct][:, jO:jO + NT],
                                                    pb_sb[:], Alu.mult)
                            hs = sw.tile([128, NT], fp16, tag=f"hs{ct}")
                            nc.vector.tensor_tensor_scan(hs[:], dA[:], dBx[:],
                                                         0.0, Alu.mult, Alu.add)
                            yt = sw.tile([128, NT], fp16, tag=f"yt{ct}")
                            nc.vector.tensor_tensor(yt[:], hs[:], pc_sb[:], Alu.mult)
                            for q in range(2):
                                nc.tensor.matmul(acc[(ct, q)][:], identsb[:],
                                                 yt[:, q * TC:(q + 1) * TC],
                                                 start=False, stop=(n == DS - 1))
                            if ct == 0 and jc == 0:
                                for f in sched.get(n, ()):
                                    f()
                    # ---- gate this half with silu(z). acc banks are copied
                    # to SBUF fp16 by Scalar first so the DVE gate multiplies
                    # run in 2x packed mode instead of the 1x PSUM mode ----
                    for ct in range(3):
                        ya = sw.tile([128, NT], fp16, tag=f"ya{ct}")
                        for q in range(2):
                            nc.scalar.copy(ya[:, q * TC:(q + 1) * TC],
                                           acc[(ct, q)][:])
                        sz3 = siluz[ct][:].rearrange("p (s l) -> p s l", l=L)
                        if jc == 0:
                            nc.vector.tensor_tensor(gsum[ct][:], siluz[ct][:],
                                                    ya[:], Alu.mult)
                        else:
                            gr = sw.tile([128, NT], fp16, tag=f"gr{ct}")
                            gr3 = gr[:].rearrange("p (s l) -> p s l", l=L)
                            nc.vector.tensor_tensor(
                                gr3[:, :, :], sz3[:, :, ::-1],
                                ya[:].rearrange("p (s l) -> p s l", l=L),
                                Alu.mult)
                            grs.append(gr)

                # ---- bidirectional sum + out_proj, chunked so the first
                # out chunk (and stage-2's readback) starts asap ----
                for j in range(2):
                    t0 = j * TC
                    for ct in range(3):
                        nc.vector.tensor_tensor(gsum[ct][:, t0:t0 + TC],
                                                gsum[ct][:, t0:t0 + TC],
                                                grs[ct][:, t0:t0 + TC], Alu.add)
                    for dchunk, dlo, dhi in ((0, 0, 128), (1, 128, 192)):
                        po = pmm.tile([dhi - dlo, TC], f32, tag="mm")
                        for ct in range(3):
                            nc.tensor.matmul(po[:], w[f"outw{ct}"][:, dlo:dhi],
                                             gsum[ct][:, t0:t0 + TC],
                                             start=(ct == 0), stop=(ct == 2))
                        if si == 0:
                            osb = w1.tile([dhi - dlo, TC], fp16, tag=f"osb{dchunk}")
                            nc.scalar.copy(osb[:], po[:])
                            o1x = o1a if j == 0 else o1b
                            dap = o1x[:, dlo:dhi, :].transpose([1, 0, 2])
                            nc.sync.dma_start(
                                dap[:, :, :],
                                osb[:].rearrange("p (s l) -> p s l", l=L))
                        else:
                            osb = w1.tile([dhi - dlo, TC], f32, tag=f"osbf{dchunk}")
                            nc.scalar.copy(osb[:], po[:])
                            nc.sync.dma_start(out_d[dlo:dhi, t0:t0 + TC], osb[:])

            emit_stage(0, wts["h"])
            emit_stage(1, wts["w"])

    split_excess_waits(nc)
    return nc


_NC_CACHE = None


def _get_nc():
    global _NC_CACHE
    if _NC_CACHE is None:
        _NC_CACHE = build_nc()
    return _NC_CACHE


def build_in_maps(inputs):
    inputs = {k: np.asarray(v, dtype=np.float32) for k, v in inputs.items()}
    x = inputs["x"]
    h16 = lambda a: np.ascontiguousarray(a.astype(np.float16))

    selB = np.zeros((DR + 2 * DS, DS * 128), np.float16)
    selC = np.zeros((DR + 2 * DS, DS * 128), np.float16)
    for n in range(DS):
        selB[DR + n, n * 128:(n + 1) * 128] = 1.0
        selC[DR + DS + n, n * 128:(n + 1) * 128] = 1.0
    base = {"ident": np.eye(128, dtype=np.float16),
            "selB": selB, "selC": selC}
    for p, tag in (("h", "h_"), ("w", "w_")):
        inw = inputs[tag + "in_proj_w"].T                        # [192, 768]
        base[f"{p}_inw1"] = h16(inw[0:128, :])
        base[f"{p}_inw2"] = h16(inw[128:192, :])
        base[f"{p}_xpw"] = h16(inputs[tag + "x_proj_w"].T)
        base[f"{p}_dtw"] = h16(inputs[tag + "dt_proj_w"].T)
        base[f"{p}_outw"] = h16(inputs[tag + "out_proj_w"].T)
        cw = inputs[tag + "conv_w"]                              # [384, 3]
        cdiag = np.zeros((DI, 3 * 128), np.float16)
        for ct in range(3):
            for k in range(3):
                cdiag[ct * 128:(ct + 1) * 128, k * 128:(k + 1) * 128] = \
                    np.diag(cw[ct * 128:(ct + 1) * 128, k].astype(np.float16))
        base[f"{p}_cdiag"] = cdiag
        dsk = inputs[tag + "D_skip"]
        ddiag = np.zeros((DI, 128), np.float16)
        for ct in range(3):
            ddiag[ct * 128:(ct + 1) * 128, :] = \
                np.diag(dsk[ct * 128:(ct + 1) * 128].astype(np.float16))
        base[f"{p}_ddiag"] = ddiag
        base[f"{p}_convb"] = inputs[tag + "conv_b"].reshape(DI, 1).copy()
        base[f"{p}_dtb"] = inputs[tag + "dt_proj_b"].reshape(DI, 1).copy()
        base[f"{p}_A"] = np.ascontiguousarray(-np.exp(inputs[tag + "A_log"]))

    in_maps = []
    for core in range(NCORE):
        sl = range(core * SPC, (core + 1) * SPC)
        seqs = np.stack([x[s // W, :, s % W, :] for s in sl])     # [14, 56, 192]
        xt = np.ascontiguousarray(seqs.reshape(NT, DM).T)         # [192, 784]
        m = dict(base)
        m["x1"] = h16(xt[0:128, :])
        m["x2"] = h16(xt[128:192, :])
        in_maps.append(m)
    return in_maps


def kernel(**inputs):
    in_maps = build_in_maps(inputs)
    nc = _get_nc()
    res = run_bass_kernel_spmd(nc, in_maps, core_ids=list(range(NCORE)))

    out_full = np.zeros((NSEQ, L, DM), np.float32)
    for core in range(NCORE):
        o = np.asarray(res.results[core]["out"], dtype=np.float32)   # [192, 784]
        out_full[core * SPC:(core + 1) * SPC] = o.T.reshape(SPC, L, DM)
    return out_full.reshape(B, H, W, DM)
